# revision 47
# baseline (speedup 1.0000x reference)
"""Trainium2 Bass kernel: dense-CRF mean-field layer (96x96 image, 21 labels).

Strategy (8 NeuronCores, row-sharded, K-stationary form):
  * Bilateral kernel K_bl [N,N] is built once on-device (fused feature matmul
    + exp) in bf16 and stays SBUF-resident per core as its [all j, own i]
    slice.
  * The per-iteration bilateral message uses K_bl tiles as the STATIONARY
    matmul operand and streams q chunks [128, 22] as the moving operand:
    out[96 own-pixels, 22] accumulates over 72 j-chunks.  Output is
    pixel-major, so the softmax/combine chain runs directly on [x, y, l]
    tiles with no transposes.
  * Spatial kernel is separable: y-blur is done with t1 (x-blurred q from the
    previous iteration, all-gathered) as the stationary operand per label,
    x-blur per own row after the softmax.  W_SPATIAL/norm folded into the
    host-prepared blur matrices.
  * Per iteration the new q is written straight into the all-gather payload
    (6 partition-shift DMA pieces), together with the x-blurred t1.
"""
import sys
sys.path.insert(0, "/opt/trn_rl_repo")
import os
import numpy as np
import ml_dtypes

H = W = 96
N = H * W                  # 9216
L = 21
LE = L + 1                 # 22 channels (21 labels + norm channel)
ALPHA, BETA, GAMMA = 80.0, 13.0, 3.0
W_SPATIAL, W_BILATERAL = 3.0, 10.0
NUM_ITERATIONS = 5
NCORES = 8
S = N // NCORES            # 1152 rows per core
YPC = H // NCORES          # 12 image rows per core
CH = N // 128              # 72 chunks of 128 rows (global j)
KCOLS = CH * S             # 82944 K_bl sbuf columns (bf16)
QCOLS = CH * LE            # 1584
PAYQ_F32 = 128 * 9 * LE // 2   # 12672 f32 slots holding the bf16 q-part
PAYT = S * LE // 2             # 12672 f32 slots holding the bf16 t1 part
PAY = PAYQ_F32 + PAYT          # 25344
ONESV = 0.1                # q norm-channel value => reciprocal gives 10/norm

# Schraudolph bf16 exp: bits_u16 = trunc(A_SCH * max(x + SH_SCH, 0)),
# bitcast as bf16 ~= exp(x) (max rel err 3.3%, C=5 fitted numerically)
A_SCH = 128.0 / 0.6931471805599453
SH_SCH = (16256.0 - 5.0) / A_SCH

# partition-shift piece groups for the 96->128 repack of q into the payload:
# maps qyb[x0:x0+n, yi, yo, l] -> q128[p0:p0+n, ao, yi, l]  (ai == yi)
# constraint: yo*96 == ao*128 - x0 + p0
QPIECES = ((0, 96, 0, 0, 0),
           (0, 96, 32, 3, 2),
           (0, 32, 96, 1, 0),
           (32, 64, 0, 1, 1),
           (0, 64, 64, 2, 1),
           (64, 32, 0, 2, 2))

LAST_EXEC_NS = None
_CACHE = {}


def _build_bass(sim1=False):
    """Build the kernel. sim1=True builds a single-core variant where the
    AllGather is replaced by 8 local DRAM copies (for TimelineSim analysis)."""
    key = "nc_sim1" if sim1 else "nc"
    if key in _CACHE:
        return _CACHE[key]
    import concourse.bass as bass  # noqa: F401
    from concourse import bacc
    import concourse.mybir as mybir
    import concourse.tile as tile

    f32 = mybir.dt.float32
    bf16 = mybir.dt.bfloat16
    AF = mybir.ActivationFunctionType
    OP = mybir.AluOpType
    AX = mybir.AxisListType

    dbg = bool(int(os.environ.get("CRF_DEBUG", "0"))) and not sim1
    nc = bacc.Bacc("TRN2", target_bir_lowering=False, debug=False,
                   num_devices=1 if sim1 else NCORES)

    featL_d = nc.dram_tensor("featL", [21, N], bf16, kind="ExternalInput")
    featR_d = nc.dram_tensor("featR", [21, S], bf16, kind="ExternalInput")
    uSB_d = nc.dram_tensor("uSB", [W, YPC * L], f32, kind="ExternalInput")
    Ax_d = nc.dram_tensor("Ax", [W, W], bf16, kind="ExternalInput")
    Ay_d = nc.dram_tensor("Ay", [H, YPC], bf16, kind="ExternalInput")
    qsb0_d = nc.dram_tensor("qsb0", [128, QCOLS], bf16, kind="ExternalInput")
    t1f0_d = nc.dram_tensor("t1f0", [H, LE * W], bf16, kind="ExternalInput")
    qout_d = nc.dram_tensor("qout", [S, L], f32, kind="ExternalOutput")
    if dbg:
        dbg_kbl = nc.dram_tensor("dbg_kbl", [128, S], bf16, kind="ExternalOutput")
        dbg_pbl = nc.dram_tensor("dbg_pbl", [W, YPC * LE], f32, kind="ExternalOutput")
        dbg_v = nc.dram_tensor("dbg_v", [W, YPC * L], f32, kind="ExternalOutput")
        dbg_lg = nc.dram_tensor("dbg_lg", [W, YPC * L], f32, kind="ExternalOutput")
        dbg_qy = nc.dram_tensor("dbg_qy", [W, YPC * L], f32, kind="ExternalOutput")
        dbg_t1t = nc.dram_tensor("dbg_t1t", [W, YPC * LE], bf16, kind="ExternalOutput")

    # combine groups: rows with y % 4 == yo finish together so the payload
    # piece(s) for that yo can fire while later groups still accumulate
    YGROUPS = [(yo, [yo, yo + 4, yo + 8]) for yo in range(4)]

    with tile.TileContext(nc) as tc:
        with (
            tc.tile_pool(name="const", bufs=1) as constp,
            tc.tile_pool(name="kbl", bufs=1) as kblp,
            tc.tile_pool(name="work", bufs=1) as work,
            tc.tile_pool(name="dram", bufs=2, space="DRAM") as dram,
        ):
            Ax = constp.tile([W, W], bf16)
            Ay = constp.tile([H, YPC], bf16)
            uSB = constp.tile([W, YPC * L], f32)
            Kbl = kblp.tile([128, KCOLS], bf16)
            qsb = work.tile([128, QCOLS], bf16, tag="qsb", bufs=2)
            t1full = work.tile([H, LE * W], bf16, tag="t1full", bufs=2)

            # ---------- precompute K_bl = exp(-||g_i - g_j||^2 / 2) ----------
            # Elementwise exp split across ACT (table exp) and DVE/Pool
            # (Schraudolph bf16-bitcast exp) over 512-col PSUM windows; the
            # 8-deep window ring lets all three engines run concurrently.
            u16 = mybir.dt.uint16
            # A: ACT table-exp from PSUM.  P: DVE shift+clamp PSUM->SBUF f32,
            # then Pool scale+u16-convert SBUF->SBUF (GPSIMD can't touch
            # PSUM).  D: DVE does both steps.
            WPAT = "AADAADAADAADAADA"  # ACT/DVE only (Pool u16 op hangs device)
            with (
                tc.tile_pool(name="pre_sb", bufs=2) as pre_sb,
                tc.tile_pool(name="stg", bufs=2) as stgp,
                tc.tile_pool(name="featRp", bufs=1) as featRp,
                tc.tile_pool(name="pre_ps", bufs=8, space="PSUM") as pre_ps,
            ):
                featR = featRp.tile([21, S], bf16)
                nc.sync.dma_start(featR[:], featR_d[:])
                flb, flb_idx = None, -1
                NW = KCOLS // 512
                for wdx in range(NW):
                    if wdx == 1:
                        # late-need loads, queued behind featR + first flb
                        nc.sync.dma_start(qsb[:], qsb0_d[:])
                        nc.sync.dma_start(t1full[:], t1f0_d[:])
                    if wdx == 2:
                        nc.sync.dma_start(Ax[:], Ax_d[:])
                        nc.sync.dma_start(Ay[:], Ay_d[:])
                        nc.sync.dma_start(uSB[:], uSB_d[:])
                    g0 = wdx * 512
                    d2 = pre_ps.tile([128, 512], f32, tag="d2")
                    a = g0
                    while a < g0 + 512:
                        ch = a // S
                        b = min(g0 + 512, (ch + 1) * S)
                        if ch // 8 != flb_idx:
                            flb_idx = ch // 8
                            flb = pre_sb.tile([21, 1024], bf16, tag="fl")
                            nc.sync.dma_start(
                                flb[:],
                                featL_d[:, flb_idx * 1024:(flb_idx + 1) * 1024])
                        nc.tensor.matmul(
                            d2[:, a - g0:b - g0],
                            flb[:, (ch % 8) * 128:(ch % 8 + 1) * 128],
                            featR[:, a - ch * S:b - ch * S],
                            start=True, stop=True)
                        a = b
                    e = WPAT[wdx % len(WPAT)]
                    if e == "A":
                        nc.scalar.activation(Kbl[:, g0:g0 + 512],
                                             d2[:, 0:512], AF.Exp)
                    elif e == "D":
                        nc.vector.tensor_scalar(d2[:, 0:512], d2[:, 0:512],
                                                SH_SCH, 0.0,
                                                op0=OP.add, op1=OP.max)
                        nc.vector.tensor_scalar(
                            Kbl[:, g0:g0 + 512].bitcast(u16),
                            d2[:, 0:512], A_SCH, None, op0=OP.mult)
                    else:
                        stg = stgp.tile([128, 512], f32, tag="stg")
                        nc.vector.tensor_scalar(stg[:], d2[:, 0:512],
                                                SH_SCH, 0.0,
                                                op0=OP.add, op1=OP.max)
                        nc.gpsimd.tensor_scalar(
                            Kbl[:, g0:g0 + 512].bitcast(u16),
                            stg[:], A_SCH, None, op0=OP.mult)

            if dbg:
                nc.sync.dma_start(dbg_kbl.ap(), Kbl[:, 0:S])

            # ---------- mean-field iterations ----------
            psBL_ctx = tc.tile_pool(name="psBL", bufs=1, space="PSUM")
            psBL = psBL_ctx.__enter__()
            psSP_ctx = tc.tile_pool(name="psSP", bufs=1, space="PSUM")
            psSP = psSP_ctx.__enter__()
            psXB_ctx = tc.tile_pool(name="psXB", bufs=2, space="PSUM")
            psXB = psXB_ctx.__enter__()
            psWM_ctx = tc.tile_pool(name="psWM", bufs=1, space="PSUM")
            psWM = psWM_ctx.__enter__()

            def pe_warm_fillers(n, dep_kbl=False):
                """Dummy 512-col matmuls that keep the tensor engine's
                p-state ramp alive across DMA-bound stretches.  With
                dep_kbl, filler k reads a late K_bl window so the stream
                paces itself to the end of the build."""
                wm = psWM.tile([LE, 512], f32, tag="warm")
                for k in range(n):
                    w = (NW - n + k) if dep_kbl else (k % 64)
                    nc.tensor.matmul(wm[:], qsb[:, 0:LE],
                                     Kbl[:, w * 512:(w + 1) * 512],
                                     start=True, stop=True)

            # pe_warm_fillers(12, dep_kbl=True)
            qag_prev = None
            for it in range(NUM_ITERATIONS):
                last = it == NUM_ITERATIONS - 1
                if it > 0:
                    qsb = work.tile([128, QCOLS], bf16, tag="qsb", bufs=2)
                    t1full = work.tile([H, LE * W], bf16, tag="t1full",
                                       bufs=2)
                    tsrc = (qag_prev[:, PAYQ_F32:PAY].bitcast(bf16)
                            .rearrange("r (y c) -> r y c", y=YPC))
                    nc.sync.dma_start(
                        t1full[:].rearrange("(r y) c -> r y c", y=YPC), tsrc)
                    for h in (0, 1):
                        qsrc = (qag_prev[h * 4:(h + 1) * 4, 0:PAYQ_F32]
                                .bitcast(bf16)
                                .rearrange("r (p c) -> p r c", p=128))
                        nc.sync.dma_start(
                            qsb[:].rearrange("p (r c) -> p r c", r=NCORES)
                            [:, h * 4:(h + 1) * 4], qsrc)

                # spatial y-blur: per label, strided t1 slice stationary
                sp = psSP.tile([W, L * YPC], f32, tag="sp", bufs=2)
                t1v = t1full[:].rearrange("Y (x l) -> Y l x", l=LE)
                for lb in range(L):
                    nc.tensor.matmul(sp[:, lb * YPC:(lb + 1) * YPC],
                                     t1v[:, lb], Ay[:],
                                     start=True, stop=True)
                u3 = uSB[:].rearrange("x (y l) -> x y l", l=L)
                sp3 = sp[:].rearrange("x (l y) -> x y l", l=L)
                pbl = psBL.tile([W, YPC * LE], f32, tag="pbl", bufs=2)

                lg = work.tile([W, YPC * L], f32, tag="lg", bufs=2)
                lg3 = lg[:].rearrange("x (y l) -> x y l", l=L)
                qy = work.tile([W, YPC * L], f32, tag="qy", bufs=2)
                qy3 = qy[:].rearrange("x (y l) -> x y l", l=L)
                ssum = work.tile([W, YPC], f32, tag="ssum", bufs=2)
                rec = work.tile([W, YPC], f32, tag="rec", bufs=2)
                if last:
                    qyf = work.tile([W, YPC * LE], f32, tag="qyf")
                    qyf3 = qyf[:].rearrange("x (y l) -> x y l", l=LE)
                else:
                    # per-group q tiles keep the payload-piece DMA deps
                    # narrow (whole-tile tracking would defer every piece
                    # to the last group's softmax)
                    qybs = [work.tile([W, 3 * LE], bf16, tag=f"qyb{g}",
                                      name=f"qyb{g}", bufs=2)
                            for g in range(4)]
                    t1X = work.tile([W, YPC * LE], bf16, tag="t1X", bufs=2)
                    xps = psXB.tile([W, YPC * LE], f32, tag="xb")
                    pl = dram.tile([1, PAY], f32, tag="pl")
                    plq = (pl[0:1, 0:PAYQ_F32].bitcast(bf16)
                           .rearrange("a (p ai ao l) -> (a p) ao ai l",
                                      p=128, ai=3, ao=3))
                pbl3 = pbl[:].rearrange("x (y l) -> x y l", l=LE)

                def emit_xblur(yo):
                    # x-blur rows y%4==yo (Ax stationary, q moving):
                    # out[x_out, l] per row; output stays x-partitioned
                    gsl = slice(yo, yo + 9, 4)
                    for k, r in enumerate((yo, yo + 4, yo + 8)):
                        nc.tensor.matmul(xps[:, r * LE:(r + 1) * LE],
                                         Ax[:],
                                         qybs[yo][:, k * LE:(k + 1) * LE],
                                         start=True, stop=True)
                    nc.scalar.copy(
                        t1X[:].rearrange("x (y l) -> x y l", l=LE)[:, gsl],
                        xps[:].rearrange("x (y l) -> x y l", l=LE)[:, gsl])

                for (yo, ys) in YGROUPS:
                    for r in ys:
                        for ch in range(CH):
                            nc.tensor.matmul(
                                pbl[:, r * LE:(r + 1) * LE],
                                Kbl[:, ch * S + r * W: ch * S + (r + 1) * W],
                                qsb[:, ch * LE:(ch + 1) * LE],
                                start=(ch == 0), stop=(ch == CH - 1))
                    # previous group's x-blur: emitted here so the PE never
                    # waits on the previous group's softmax chain
                    if yo > 0 and not last:
                        emit_xblur(yo - 1)
                    # combine + softmax for this group's 3 rows
                    gsl = slice(yo, yo + 9, 4)  # rows yo, yo+4, yo+8
                    nc.vector.reciprocal(rec[:, gsl][:, :, None],
                                         pbl3[:, gsl, L:LE])
                    nc.vector.tensor_tensor(
                        lg3[:, gsl], pbl3[:, gsl, 0:L],
                        rec[:, gsl][:, :, None].to_broadcast([W, 3, L]),
                        OP.mult)
                    nc.vector.tensor_tensor(lg3[:, gsl], lg3[:, gsl],
                                            u3[:, gsl], OP.add)
                    nc.vector.tensor_tensor(lg3[:, gsl], lg3[:, gsl],
                                            sp3[:, gsl], OP.add)
                    nc.scalar.activation(qy3[:, gsl], lg3[:, gsl], AF.Exp)
                    nc.vector.reduce_sum(ssum[:, gsl], qy3[:, gsl], axis=AX.X)
                    nc.vector.reciprocal(ssum[:, gsl], ssum[:, gsl])
                    qt = (qyf3[:, gsl] if last
                          else qybs[yo][:].rearrange("x (k l) -> x k l", l=LE))
                    nc.vector.tensor_tensor(
                        qt[:, :, 0:L], qy3[:, gsl],
                        ssum[:, gsl][:, :, None].to_broadcast([W, 3, L]),
                        OP.mult)
                    if last:
                        continue
                    if it < 2:
                        nc.vector.memset(qt[:, :, L:LE], ONESV)
                    # payload piece(s) for this yo straight into DRAM, on
                    # the otherwise-idle SWDGE queue
                    for (x0, n, p0, pyo, ao) in QPIECES:
                        if pyo == yo:
                            nc.gpsimd.dma_start(
                                plq[p0:p0 + n, ao],
                                qybs[yo][x0:x0 + n, :]
                                .rearrange("x (k l) -> x k l", l=LE))
                if not last:
                    emit_xblur(3)

                if dbg and it == 0:
                    nc.sync.dma_start(dbg_pbl.ap(), pbl[:])
                    nc.sync.dma_start(dbg_lg.ap(), lg[:])
                if dbg and it == 1:
                    nc.sync.dma_start(dbg_qy.ap(), qy[:])
                    nc.sync.dma_start(dbg_t1t.ap(), t1X[:])

                if last:
                    nc.sync.dma_start(
                        qout_d.ap().rearrange("(y x) l -> x y l", x=W),
                        qyf3[:, :, 0:L])
                    continue

                # t1 payload part: dst layout (y, x, l) per core
                nc.scalar.dma_start(
                    pl[0:1, PAYQ_F32:PAY].bitcast(bf16)
                      .rearrange("a (y x l) -> (a x) y l", y=YPC, x=W, l=LE),
                    t1X[:].rearrange("x (y l) -> x y l", l=LE))
                # AllGather (sim1: two broadcast DRAM copies, same data
                # volume as 8 per-peer copies)
                qag = dram.tile([NCORES, PAY], f32, tag="qag")
                if sim1:
                    nc.sync.dma_start(qag[0:4, :],
                                      pl[:].to_broadcast([4, PAY]))
                    nc.sync.dma_start(qag[4:8, :],
                                      pl[:].to_broadcast([4, PAY]))
                else:
                    nc.gpsimd.collective_compute(
                        "AllGather", OP.bypass,
                        replica_groups=[list(range(NCORES))],
                        ins=[pl.opt()], outs=[qag.opt()])
                qag_prev = qag
            psWM_ctx.__exit__(None, None, None)
            psXB_ctx.__exit__(None, None, None)
            psSP_ctx.__exit__(None, None, None)
            psBL_ctx.__exit__(None, None, None)

    nc.compile()
    _CACHE[key] = nc
    return nc


def _host_prepare(unaries, rgb):
    u = np.asarray(unaries, np.float32).reshape(N, L)
    c = np.asarray(rgb, np.float32).reshape(N, 3)

    ys, xs = np.meshgrid(np.arange(H, dtype=np.float64),
                         np.arange(W, dtype=np.float64), indexing="ij")
    pos = np.stack([ys.ravel(), xs.ravel()], -1)            # [N, 2]
    g = np.concatenate([c.astype(np.float64) / BETA, pos / ALPHA], 1)
    g = g - g.mean(0, keepdims=True)
    sq = (g * g).sum(1)
    ones = np.ones(N, np.float64)
    L7 = np.concatenate([g.T, ones[None], (-0.5 * sq)[None]], 0)  # [7, N] j
    R7 = np.concatenate([g.T, (-0.5 * sq)[None], ones[None]], 0)  # [7, N] i
    bfd = ml_dtypes.bfloat16
    Lhi = L7.astype(bfd)
    Llo = (L7 - Lhi.astype(np.float64)).astype(bfd)
    Rhi = R7.astype(bfd)
    Rlo = (R7 - Rhi.astype(np.float64)).astype(bfd)
    # dot = Lhi.Rhi + Lhi.Rlo + Llo.Rhi  (Llo.Rlo dropped, ~1e-3)
    featL = np.ascontiguousarray(np.concatenate([Lhi, Lhi, Llo], 0))  # [21,N]
    featR = np.ascontiguousarray(np.concatenate([Rhi, Rlo, Rhi], 0))  # [21,N]

    d = np.arange(W, dtype=np.float64)
    A = np.exp(-(d[:, None] - d[None, :]) ** 2 / (2.0 * GAMMA * GAMMA))
    nvec = A.sum(0)
    Ax = np.ascontiguousarray((A / nvec[None, :]).astype(ml_dtypes.bfloat16))

    um = u.max(1, keepdims=True)
    e = np.exp(u - um)
    q0 = e / e.sum(1, keepdims=True)
    q0e = np.concatenate([q0, np.full((N, 1), ONESV, np.float32)], 1)  # [N,22]
    qsb0 = np.ascontiguousarray(
        q0e.reshape(CH, 128, LE).transpose(1, 0, 2).reshape(128, QCOLS)
    ).astype(ml_dtypes.bfloat16)

    q3 = q0e.reshape(H, W, LE).astype(np.float64)
    t1 = np.einsum("Xx,yXl->yxl", A / nvec[None, :], q3)      # [96, 96, 22]
    t1f0 = np.ascontiguousarray(t1.reshape(H, W * LE).astype(ml_dtypes.bfloat16))

    in_maps = []
    for core in range(NCORES):
        rows = slice(core * S, (core + 1) * S)
        uSB_c = np.ascontiguousarray(
            u[rows].reshape(YPC, W, L).transpose(1, 0, 2).reshape(W, YPC * L))
        yc = slice(core * YPC, (core + 1) * YPC)
        Ay_c = np.ascontiguousarray(
            (A[:, yc] * (W_SPATIAL / nvec[yc])[None, :]).astype(ml_dtypes.bfloat16))
        in_maps.append({
            "featL": featL,
            "featR": np.ascontiguousarray(featR[:, rows]),
            "uSB": uSB_c,
            "Ax": Ax,
            "Ay": Ay_c,
            "qsb0": qsb0,
            "t1f0": t1f0,
        })
    return in_maps


def _get_runner():
    """Compile once; return (fn, in_names, out_names) where fn maps
    concatenated global numpy inputs -> list of per-core output dicts."""
    if "runner" in _CACHE:
        return _CACHE["runner"]
    import jax
    from jax.sharding import Mesh, PartitionSpec
    from jax.experimental.shard_map import shard_map
    import concourse.mybir as mybir
    from concourse import bass2jax

    nc = _build_bass()
    bass2jax.install_neuronx_cc_hook()

    partition_name = (nc.partition_id_tensor.name
                      if nc.partition_id_tensor else None)
    in_names, out_names, out_avals, zero_outs = [], [], [], []
    for alloc in nc.m.functions[0].allocations:
        if not isinstance(alloc, mybir.MemoryLocationSet):
            continue
        name = alloc.memorylocations[0].name
        if alloc.kind == "ExternalInput":
            if name != partition_name:
                in_names.append(name)
        elif alloc.kind == "ExternalOutput":
            shape = tuple(alloc.tensor_shape)
            dtype = mybir.dt.np(alloc.dtype)
            out_names.append(name)
            out_avals.append(jax.core.ShapedArray(shape, dtype))
            zero_outs.append(np.zeros(shape, dtype))
    n_params = len(in_names)
    all_in_names = list(in_names) + list(out_names)
    if partition_name is not None:
        all_in_names.append(partition_name)

    def _body(*args):
        operands = list(args)
        if partition_name is not None:
            operands.append(bass2jax.partition_id_tensor())
        outs = bass2jax._bass_exec_p.bind(
            *operands,
            out_avals=tuple(out_avals),
            in_names=tuple(all_in_names),
            out_names=tuple(out_names),
            lowering_input_output_aliases=(),
            sim_require_finite=False,
            sim_require_nnan=False,
            nc=nc,
        )
        return tuple(outs)

    devices = jax.devices()[:NCORES]
    mesh = Mesh(np.asarray(devices), ("core",))
    n_outs = len(out_names)
    in_specs = (PartitionSpec("core"),) * (n_params + n_outs)
    out_specs = (PartitionSpec("core"),) * n_outs
    donate = tuple(range(n_params, n_params + n_outs))
    fn = jax.jit(
        shard_map(_body, mesh=mesh, in_specs=in_specs, out_specs=out_specs,
                  check_rep=False),
        donate_argnums=donate, keep_unused=True)
    _CACHE["runner"] = (fn, in_names, out_names, out_avals, zero_outs)
    return _CACHE["runner"]


def _concat_inputs(in_maps, in_names):
    return [np.concatenate([np.asarray(in_maps[c][nm]) for c in range(NCORES)],
                           axis=0) for nm in in_names]


def _run(in_maps):
    fn, in_names, out_names, out_avals, zero_outs = _get_runner()
    concat_in = _concat_inputs(in_maps, in_names)
    concat_zeros = [np.zeros((NCORES * z.shape[0], *z.shape[1:]), z.dtype)
                    for z in zero_outs]
    out_arrs = fn(*concat_in, *concat_zeros)
    return out_arrs, out_names, out_avals


def kernel(unaries, rgb):
    in_maps = _host_prepare(unaries, rgb)
    out_arrs, out_names, out_avals = _run(in_maps)
    qi = out_names.index("qout")
    q = np.asarray(out_arrs[qi]).reshape(NCORES, S, L).reshape(N, L)
    return np.ascontiguousarray(q[None].astype(np.float32))


def time_kernel(unaries, rgb, iters=20):
    """Steady-state per-call wall time of the compiled 8-core executable,
    with inputs pre-staged on device."""
    import time as _time
    import jax
    in_maps = _host_prepare(unaries, rgb)
    fn, in_names, out_names, out_avals, zero_outs = _get_runner()
    concat_in = _concat_inputs(in_maps, in_names)

    def once():
        concat_zeros = [np.zeros((NCORES * z.shape[0], *z.shape[1:]), z.dtype)
                        for z in zero_outs]
        outs = fn(*concat_in, *concat_zeros)
        jax.block_until_ready(outs)
        return outs

    once()  # warm
    times = []
    for _ in range(iters):
        t0 = _time.perf_counter()
        once()
        times.append(_time.perf_counter() - t0)
    return min(times), sorted(times)[len(times) // 2]


# revision 49
# speedup vs baseline: 1.5065x; 1.5065x over previous
"""Trainium2 Bass kernel: dense-CRF mean-field layer (96x96 image, 21 labels).

Strategy (8 NeuronCores, row-sharded, K-stationary form):
  * Bilateral kernel K_bl [N,N] is built once on-device (fused feature matmul
    + exp) in bf16 and stays SBUF-resident per core as its [all j, own i]
    slice.
  * The per-iteration bilateral message uses K_bl tiles as the STATIONARY
    matmul operand and streams q chunks [128, 22] as the moving operand:
    out[96 own-pixels, 22] accumulates over 72 j-chunks.  Output is
    pixel-major, so the softmax/combine chain runs directly on [x, y, l]
    tiles with no transposes.
  * Spatial kernel is separable: y-blur is done with t1 (x-blurred q from the
    previous iteration, all-gathered) as the stationary operand per label,
    x-blur per own row after the softmax.  W_SPATIAL/norm folded into the
    host-prepared blur matrices.
  * Per iteration the new q is written straight into the all-gather payload
    (6 partition-shift DMA pieces), together with the x-blurred t1.
"""
import sys
sys.path.insert(0, "/opt/trn_rl_repo")
import os
import numpy as np
import ml_dtypes

H = W = 96
N = H * W                  # 9216
L = 21
LE = L + 1                 # 22 channels (21 labels + norm channel)
ALPHA, BETA, GAMMA = 80.0, 13.0, 3.0
W_SPATIAL, W_BILATERAL = 3.0, 10.0
NUM_ITERATIONS = 5
NCORES = 8
S = N // NCORES            # 1152 rows per core
YPC = H // NCORES          # 12 image rows per core
CH = N // 128              # 72 chunks of 128 rows (global j)
KCOLS = CH * S             # 82944 K_bl sbuf columns (bf16)
QCOLS = CH * LE            # 1584
PAYQ_F32 = 128 * 9 * LE // 2   # 12672 f32 slots holding the bf16 q-part
PAYT = S * LE // 2             # 12672 f32 slots holding the bf16 t1 part
PAY = PAYQ_F32 + PAYT          # 25344
ONESV = 0.1                # q norm-channel value => reciprocal gives 10/norm

# Schraudolph bf16 exp: bits_u16 = trunc(A_SCH * max(x + SH_SCH, 0)),
# bitcast as bf16 ~= exp(x) (max rel err 3.3%, C=5 fitted numerically)
A_SCH = 128.0 / 0.6931471805599453
SH_SCH = (16256.0 - 5.0) / A_SCH

# partition-shift piece groups for the 96->128 repack of q into the payload:
# maps qyb[x0:x0+n, yi, yo, l] -> q128[p0:p0+n, ao, yi, l]  (ai == yi)
# constraint: yo*96 == ao*128 - x0 + p0
QPIECES = ((0, 96, 0, 0, 0),
           (0, 96, 32, 3, 2),
           (0, 32, 96, 1, 0),
           (32, 64, 0, 1, 1),
           (0, 64, 64, 2, 1),
           (64, 32, 0, 2, 2))

LAST_EXEC_NS = None
_CACHE = {}


def _build_bass(sim1=False):
    """Build the kernel. sim1=True builds a single-core variant where the
    AllGather is replaced by 8 local DRAM copies (for TimelineSim analysis)."""
    key = "nc_sim1" if sim1 else "nc"
    if key in _CACHE:
        return _CACHE[key]
    import concourse.bass as bass  # noqa: F401
    from concourse import bacc
    import concourse.mybir as mybir
    import concourse.tile as tile

    f32 = mybir.dt.float32
    bf16 = mybir.dt.bfloat16
    AF = mybir.ActivationFunctionType
    OP = mybir.AluOpType
    AX = mybir.AxisListType

    dbg = bool(int(os.environ.get("CRF_DEBUG", "0"))) and not sim1
    nc = bacc.Bacc("TRN2", target_bir_lowering=False, debug=False,
                   num_devices=1 if sim1 else NCORES)

    featL_d = nc.dram_tensor("featL", [21, N], bf16, kind="ExternalInput")
    featR_d = nc.dram_tensor("featR", [21, S], bf16, kind="ExternalInput")
    uSB_d = nc.dram_tensor("uSB", [W, YPC * L], f32, kind="ExternalInput")
    Ax_d = nc.dram_tensor("Ax", [W, W], bf16, kind="ExternalInput")
    Ay_d = nc.dram_tensor("Ay", [H, YPC], bf16, kind="ExternalInput")
    qsb0_d = nc.dram_tensor("qsb0", [128, QCOLS], bf16, kind="ExternalInput")
    t1f0_d = nc.dram_tensor("t1f0", [H, LE * W], bf16, kind="ExternalInput")
    qout_d = nc.dram_tensor("qout", [S, L], f32, kind="ExternalOutput")
    if dbg:
        dbg_kbl = nc.dram_tensor("dbg_kbl", [128, S], bf16, kind="ExternalOutput")
        dbg_pbl = nc.dram_tensor("dbg_pbl", [W, YPC * LE], f32, kind="ExternalOutput")
        dbg_v = nc.dram_tensor("dbg_v", [W, YPC * L], f32, kind="ExternalOutput")
        dbg_lg = nc.dram_tensor("dbg_lg", [W, YPC * L], f32, kind="ExternalOutput")
        dbg_qy = nc.dram_tensor("dbg_qy", [W, YPC * L], f32, kind="ExternalOutput")
        dbg_t1t = nc.dram_tensor("dbg_t1t", [LE, S], bf16, kind="ExternalOutput")

    # combine groups: rows with y % 4 == yo finish together so the payload
    # piece(s) for that yo can fire while later groups still accumulate
    YGROUPS = [(yo, [yo, yo + 4, yo + 8]) for yo in range(4)]

    with tile.TileContext(nc) as tc:
        with (
            tc.tile_pool(name="const", bufs=1) as constp,
            tc.tile_pool(name="kbl", bufs=1) as kblp,
            tc.tile_pool(name="work", bufs=1) as work,
            tc.tile_pool(name="dram", bufs=2, space="DRAM") as dram,
        ):
            Ax = constp.tile([W, W], bf16)
            Ay = constp.tile([H, YPC], bf16)
            uSB = constp.tile([W, YPC * L], f32)
            Kbl = kblp.tile([128, KCOLS], bf16)
            qsb = work.tile([128, QCOLS], bf16, tag="qsb", bufs=2)
            t1full = work.tile([H, LE * W], bf16, tag="t1full", bufs=2)

            # ---------- precompute K_bl = exp(-||g_i - g_j||^2 / 2) ----------
            # Elementwise exp split across ACT (table exp) and DVE/Pool
            # (Schraudolph bf16-bitcast exp) over 512-col PSUM windows; the
            # 8-deep window ring lets all three engines run concurrently.
            u16 = mybir.dt.uint16
            # A: ACT table-exp from PSUM.  P: DVE shift+clamp PSUM->SBUF f32,
            # then Pool scale+u16-convert SBUF->SBUF (GPSIMD can't touch
            # PSUM).  D: DVE does both steps.
            WPAT = "AAAAAAAAAAAAAAAA"  # bisect: all-ACT exp
            with (
                tc.tile_pool(name="pre_sb", bufs=2) as pre_sb,
                tc.tile_pool(name="stg", bufs=2) as stgp,
                tc.tile_pool(name="featRp", bufs=1) as featRp,
                tc.tile_pool(name="pre_ps", bufs=8, space="PSUM") as pre_ps,
            ):
                featR = featRp.tile([21, S], bf16)
                nc.sync.dma_start(featR[:], featR_d[:])
                flb, flb_idx = None, -1
                NW = KCOLS // 512
                for wdx in range(NW):
                    if wdx == 1:
                        # late-need loads, queued behind featR + first flb
                        nc.sync.dma_start(qsb[:], qsb0_d[:])
                        nc.sync.dma_start(t1full[:], t1f0_d[:])
                    if wdx == 2:
                        nc.sync.dma_start(Ax[:], Ax_d[:])
                        nc.sync.dma_start(Ay[:], Ay_d[:])
                        nc.sync.dma_start(uSB[:], uSB_d[:])
                    g0 = wdx * 512
                    d2 = pre_ps.tile([128, 512], f32, tag="d2")
                    a = g0
                    while a < g0 + 512:
                        ch = a // S
                        b = min(g0 + 512, (ch + 1) * S)
                        if ch // 8 != flb_idx:
                            flb_idx = ch // 8
                            flb = pre_sb.tile([21, 1024], bf16, tag="fl")
                            nc.sync.dma_start(
                                flb[:],
                                featL_d[:, flb_idx * 1024:(flb_idx + 1) * 1024])
                        nc.tensor.matmul(
                            d2[:, a - g0:b - g0],
                            flb[:, (ch % 8) * 128:(ch % 8 + 1) * 128],
                            featR[:, a - ch * S:b - ch * S],
                            start=True, stop=True)
                        a = b
                    e = WPAT[wdx % len(WPAT)]
                    if e == "A":
                        nc.scalar.activation(Kbl[:, g0:g0 + 512],
                                             d2[:, 0:512], AF.Exp)
                    elif e == "D":
                        nc.vector.tensor_scalar(d2[:, 0:512], d2[:, 0:512],
                                                SH_SCH, 0.0,
                                                op0=OP.add, op1=OP.max)
                        nc.vector.tensor_scalar(
                            Kbl[:, g0:g0 + 512].bitcast(u16),
                            d2[:, 0:512], A_SCH, None, op0=OP.mult)
                    else:
                        stg = stgp.tile([128, 512], f32, tag="stg")
                        nc.vector.tensor_scalar(stg[:], d2[:, 0:512],
                                                SH_SCH, 0.0,
                                                op0=OP.add, op1=OP.max)
                        nc.gpsimd.tensor_scalar(
                            Kbl[:, g0:g0 + 512].bitcast(u16),
                            stg[:], A_SCH, None, op0=OP.mult)

            if dbg:
                nc.sync.dma_start(dbg_kbl.ap(), Kbl[:, 0:S])

            # ---------- mean-field iterations ----------
            psBL_ctx = tc.tile_pool(name="psBL", bufs=1, space="PSUM")
            psBL = psBL_ctx.__enter__()
            psSP_ctx = tc.tile_pool(name="psSP", bufs=1, space="PSUM")
            psSP = psSP_ctx.__enter__()
            psXB_ctx = tc.tile_pool(name="psXB", bufs=2, space="PSUM")
            psXB = psXB_ctx.__enter__()
            psWM_ctx = tc.tile_pool(name="psWM", bufs=1, space="PSUM")
            psWM = psWM_ctx.__enter__()

            def pe_warm_fillers(n, dep_kbl=False):
                """Dummy 512-col matmuls that keep the tensor engine's
                p-state ramp alive across DMA-bound stretches.  With
                dep_kbl, filler k reads a late K_bl window so the stream
                paces itself to the end of the build."""
                wm = psWM.tile([LE, 512], f32, tag="warm")
                for k in range(n):
                    w = (NW - n + k) if dep_kbl else (k % 64)
                    nc.tensor.matmul(wm[:], qsb[:, 0:LE],
                                     Kbl[:, w * 512:(w + 1) * 512],
                                     start=True, stop=True)

            # pe_warm_fillers(12, dep_kbl=True)
            qag_prev = None
            for it in range(NUM_ITERATIONS):
                last = it == NUM_ITERATIONS - 1
                if it > 0:
                    qsb = work.tile([128, QCOLS], bf16, tag="qsb", bufs=2)
                    t1full = work.tile([H, LE * W], bf16, tag="t1full",
                                       bufs=2)
                    for r in range(NCORES):
                        tsrc = (qag_prev[r:r + 1, PAYQ_F32:PAY].bitcast(bf16)
                                .rearrange("a b -> (a b)")
                                .rearrange("(l y x) -> y l x",
                                           l=LE, y=YPC, x=W))
                        tdst = (t1full[r * YPC:(r + 1) * YPC, :]
                                .rearrange("y (l x) -> y l x", l=LE, x=W))
                        eng = nc.sync if r % 2 else nc.gpsimd
                        eng.dma_start(tdst, tsrc)
                    for h in (0, 1):
                        qsrc = (qag_prev[h * 4:(h + 1) * 4, 0:PAYQ_F32]
                                .bitcast(bf16)
                                .rearrange("r (p c) -> p r c", p=128))
                        nc.sync.dma_start(
                            qsb[:].rearrange("p (r c) -> p r c", r=NCORES)
                            [:, h * 4:(h + 1) * 4], qsrc)

                # spatial y-blur: per label, strided t1 slice stationary
                sp = psSP.tile([W, L * YPC], f32, tag="sp", bufs=2)
                for lb in range(L):
                    nc.tensor.matmul(sp[:, lb * YPC:(lb + 1) * YPC],
                                     t1full[:, lb * W:(lb + 1) * W], Ay[:],
                                     start=True, stop=True)
                u3 = uSB[:].rearrange("x (y l) -> x y l", l=L)
                sp3 = sp[:].rearrange("x (l y) -> x y l", l=L)
                pbl = psBL.tile([W, YPC * LE], f32, tag="pbl", bufs=2)

                lg = work.tile([W, YPC * L], f32, tag="lg", bufs=2)
                lg3 = lg[:].rearrange("x (y l) -> x y l", l=L)
                qy = work.tile([W, YPC * L], f32, tag="qy", bufs=2)
                qy3 = qy[:].rearrange("x (y l) -> x y l", l=L)
                ssum = work.tile([W, YPC], f32, tag="ssum", bufs=2)
                rec = work.tile([W, YPC], f32, tag="rec", bufs=2)
                if last:
                    qyf = work.tile([W, YPC * LE], f32, tag="qyf")
                    qyf3 = qyf[:].rearrange("x (y l) -> x y l", l=LE)
                else:
                    # per-group q tiles keep the payload-piece DMA deps
                    # narrow (whole-tile tracking would defer every piece
                    # to the last group's softmax)
                    qybs = [work.tile([W, 3 * LE], bf16, tag=f"qyb{g}",
                                      name=f"qyb{g}", bufs=2)
                            for g in range(4)]
                    t1X = work.tile([LE, S], bf16, tag="t1X", bufs=2)
                    pl = dram.tile([1, PAY], f32, tag="pl")
                    plq = (pl[0:1, 0:PAYQ_F32].bitcast(bf16)
                           .rearrange("a (p ai ao l) -> (a p) ao ai l",
                                      p=128, ai=3, ao=3))
                pbl3 = pbl[:].rearrange("x (y l) -> x y l", l=LE)

                def emit_xblur(yo):
                    # x-blur rows y%4==yo (new q stationary, Ax moving)
                    gsl = slice(yo, yo + 9, 4)
                    xb = psXB.tile([LE, 3 * W], f32, tag="xb")
                    for k, r in enumerate((yo, yo + 4, yo + 8)):
                        nc.tensor.matmul(xb[:, k * W:(k + 1) * W],
                                         qybs[yo][:, k * LE:(k + 1) * LE],
                                         Ax[:], start=True, stop=True)
                    nc.scalar.copy(
                        t1X[:].rearrange("l (y x) -> l y x", x=W)[:, gsl],
                        xb[:].rearrange("l (k x) -> l k x", x=W))

                for (yo, ys) in YGROUPS:
                    for r in ys:
                        for ch in range(CH):
                            nc.tensor.matmul(
                                pbl[:, r * LE:(r + 1) * LE],
                                Kbl[:, ch * S + r * W: ch * S + (r + 1) * W],
                                qsb[:, ch * LE:(ch + 1) * LE],
                                start=(ch == 0), stop=(ch == CH - 1))
                    # previous group's x-blur: emitted here so the PE never
                    # waits on the previous group's softmax chain
                    if yo > 0 and not last:
                        emit_xblur(yo - 1)
                    # combine + softmax for this group's 3 rows
                    gsl = slice(yo, yo + 9, 4)  # rows yo, yo+4, yo+8
                    nc.vector.reciprocal(rec[:, gsl][:, :, None],
                                         pbl3[:, gsl, L:LE])
                    nc.vector.tensor_tensor(
                        lg3[:, gsl], pbl3[:, gsl, 0:L],
                        rec[:, gsl][:, :, None].to_broadcast([W, 3, L]),
                        OP.mult)
                    nc.vector.tensor_tensor(lg3[:, gsl], lg3[:, gsl],
                                            u3[:, gsl], OP.add)
                    nc.vector.tensor_tensor(lg3[:, gsl], lg3[:, gsl],
                                            sp3[:, gsl], OP.add)
                    nc.scalar.activation(qy3[:, gsl], lg3[:, gsl], AF.Exp)
                    nc.vector.reduce_sum(ssum[:, gsl], qy3[:, gsl], axis=AX.X)
                    nc.vector.reciprocal(ssum[:, gsl], ssum[:, gsl])
                    qt = (qyf3[:, gsl] if last
                          else qybs[yo][:].rearrange("x (k l) -> x k l", l=LE))
                    nc.vector.tensor_tensor(
                        qt[:, :, 0:L], qy3[:, gsl],
                        ssum[:, gsl][:, :, None].to_broadcast([W, 3, L]),
                        OP.mult)
                    if last:
                        continue
                    if it < 2:
                        nc.vector.memset(qt[:, :, L:LE], ONESV)
                    # payload piece(s) for this yo straight into DRAM, on
                    # the otherwise-idle SWDGE queue
                    for (x0, n, p0, pyo, ao) in QPIECES:
                        if pyo == yo:
                            nc.gpsimd.dma_start(
                                plq[p0:p0 + n, ao],
                                qybs[yo][x0:x0 + n, :]
                                .rearrange("x (k l) -> x k l", l=LE))
                if not last:
                    emit_xblur(3)

                if dbg and it == 0:
                    nc.sync.dma_start(dbg_pbl.ap(), pbl[:])
                    nc.sync.dma_start(dbg_lg.ap(), lg[:])
                if dbg and it == 1:
                    nc.sync.dma_start(dbg_qy.ap(), qy[:])
                    nc.sync.dma_start(dbg_t1t.ap(), t1X[:])

                if last:
                    nc.sync.dma_start(
                        qout_d.ap().rearrange("(y x) l -> x y l", x=W),
                        qyf3[:, :, 0:L])
                    continue

                # t1 payload part (layout (l, y, x) per core)
                nc.scalar.dma_start(
                    pl[0:1, PAYQ_F32:PAY].bitcast(bf16)
                      .rearrange("a (l c) -> (a l) c", l=LE),
                    t1X[:])
                # AllGather (sim1: two broadcast DRAM copies, same data
                # volume as 8 per-peer copies)
                qag = dram.tile([NCORES, PAY], f32, tag="qag")
                if sim1:
                    nc.sync.dma_start(qag[0:4, :],
                                      pl[:].to_broadcast([4, PAY]))
                    nc.sync.dma_start(qag[4:8, :],
                                      pl[:].to_broadcast([4, PAY]))
                else:
                    nc.gpsimd.collective_compute(
                        "AllGather", OP.bypass,
                        replica_groups=[list(range(NCORES))],
                        ins=[pl.opt()], outs=[qag.opt()])
                qag_prev = qag
            psWM_ctx.__exit__(None, None, None)
            psXB_ctx.__exit__(None, None, None)
            psSP_ctx.__exit__(None, None, None)
            psBL_ctx.__exit__(None, None, None)

    nc.compile()
    _CACHE[key] = nc
    return nc


def _host_prepare(unaries, rgb):
    u = np.asarray(unaries, np.float32).reshape(N, L)
    c = np.asarray(rgb, np.float32).reshape(N, 3)

    ys, xs = np.meshgrid(np.arange(H, dtype=np.float64),
                         np.arange(W, dtype=np.float64), indexing="ij")
    pos = np.stack([ys.ravel(), xs.ravel()], -1)            # [N, 2]
    g = np.concatenate([c.astype(np.float64) / BETA, pos / ALPHA], 1)
    g = g - g.mean(0, keepdims=True)
    sq = (g * g).sum(1)
    ones = np.ones(N, np.float64)
    L7 = np.concatenate([g.T, ones[None], (-0.5 * sq)[None]], 0)  # [7, N] j
    R7 = np.concatenate([g.T, (-0.5 * sq)[None], ones[None]], 0)  # [7, N] i
    bfd = ml_dtypes.bfloat16
    Lhi = L7.astype(bfd)
    Llo = (L7 - Lhi.astype(np.float64)).astype(bfd)
    Rhi = R7.astype(bfd)
    Rlo = (R7 - Rhi.astype(np.float64)).astype(bfd)
    # dot = Lhi.Rhi + Lhi.Rlo + Llo.Rhi  (Llo.Rlo dropped, ~1e-3)
    featL = np.ascontiguousarray(np.concatenate([Lhi, Lhi, Llo], 0))  # [21,N]
    featR = np.ascontiguousarray(np.concatenate([Rhi, Rlo, Rhi], 0))  # [21,N]

    d = np.arange(W, dtype=np.float64)
    A = np.exp(-(d[:, None] - d[None, :]) ** 2 / (2.0 * GAMMA * GAMMA))
    nvec = A.sum(0)
    Ax = np.ascontiguousarray((A / nvec[None, :]).astype(ml_dtypes.bfloat16))

    um = u.max(1, keepdims=True)
    e = np.exp(u - um)
    q0 = e / e.sum(1, keepdims=True)
    q0e = np.concatenate([q0, np.full((N, 1), ONESV, np.float32)], 1)  # [N,22]
    qsb0 = np.ascontiguousarray(
        q0e.reshape(CH, 128, LE).transpose(1, 0, 2).reshape(128, QCOLS)
    ).astype(ml_dtypes.bfloat16)

    q3 = q0e.reshape(H, W, LE).astype(np.float64)
    t1 = np.einsum("Xx,yXl->ylx", A / nvec[None, :], q3)      # [96, 22, 96]
    t1f0 = np.ascontiguousarray(t1.reshape(H, LE * W).astype(ml_dtypes.bfloat16))

    in_maps = []
    for core in range(NCORES):
        rows = slice(core * S, (core + 1) * S)
        uSB_c = np.ascontiguousarray(
            u[rows].reshape(YPC, W, L).transpose(1, 0, 2).reshape(W, YPC * L))
        yc = slice(core * YPC, (core + 1) * YPC)
        Ay_c = np.ascontiguousarray(
            (A[:, yc] * (W_SPATIAL / nvec[yc])[None, :]).astype(ml_dtypes.bfloat16))
        in_maps.append({
            "featL": featL,
            "featR": np.ascontiguousarray(featR[:, rows]),
            "uSB": uSB_c,
            "Ax": Ax,
            "Ay": Ay_c,
            "qsb0": qsb0,
            "t1f0": t1f0,
        })
    return in_maps


def _get_runner():
    """Compile once; return (fn, in_names, out_names) where fn maps
    concatenated global numpy inputs -> list of per-core output dicts."""
    if "runner" in _CACHE:
        return _CACHE["runner"]
    import jax
    from jax.sharding import Mesh, PartitionSpec
    from jax.experimental.shard_map import shard_map
    import concourse.mybir as mybir
    from concourse import bass2jax

    nc = _build_bass()
    bass2jax.install_neuronx_cc_hook()

    partition_name = (nc.partition_id_tensor.name
                      if nc.partition_id_tensor else None)
    in_names, out_names, out_avals, zero_outs = [], [], [], []
    for alloc in nc.m.functions[0].allocations:
        if not isinstance(alloc, mybir.MemoryLocationSet):
            continue
        name = alloc.memorylocations[0].name
        if alloc.kind == "ExternalInput":
            if name != partition_name:
                in_names.append(name)
        elif alloc.kind == "ExternalOutput":
            shape = tuple(alloc.tensor_shape)
            dtype = mybir.dt.np(alloc.dtype)
            out_names.append(name)
            out_avals.append(jax.core.ShapedArray(shape, dtype))
            zero_outs.append(np.zeros(shape, dtype))
    n_params = len(in_names)
    all_in_names = list(in_names) + list(out_names)
    if partition_name is not None:
        all_in_names.append(partition_name)

    def _body(*args):
        operands = list(args)
        if partition_name is not None:
            operands.append(bass2jax.partition_id_tensor())
        outs = bass2jax._bass_exec_p.bind(
            *operands,
            out_avals=tuple(out_avals),
            in_names=tuple(all_in_names),
            out_names=tuple(out_names),
            lowering_input_output_aliases=(),
            sim_require_finite=False,
            sim_require_nnan=False,
            nc=nc,
        )
        return tuple(outs)

    devices = jax.devices()[:NCORES]
    mesh = Mesh(np.asarray(devices), ("core",))
    n_outs = len(out_names)
    in_specs = (PartitionSpec("core"),) * (n_params + n_outs)
    out_specs = (PartitionSpec("core"),) * n_outs
    donate = tuple(range(n_params, n_params + n_outs))
    fn = jax.jit(
        shard_map(_body, mesh=mesh, in_specs=in_specs, out_specs=out_specs,
                  check_rep=False),
        donate_argnums=donate, keep_unused=True)
    _CACHE["runner"] = (fn, in_names, out_names, out_avals, zero_outs)
    return _CACHE["runner"]


def _concat_inputs(in_maps, in_names):
    return [np.concatenate([np.asarray(in_maps[c][nm]) for c in range(NCORES)],
                           axis=0) for nm in in_names]


def _run(in_maps):
    fn, in_names, out_names, out_avals, zero_outs = _get_runner()
    concat_in = _concat_inputs(in_maps, in_names)
    concat_zeros = [np.zeros((NCORES * z.shape[0], *z.shape[1:]), z.dtype)
                    for z in zero_outs]
    out_arrs = fn(*concat_in, *concat_zeros)
    return out_arrs, out_names, out_avals


def kernel(unaries, rgb):
    in_maps = _host_prepare(unaries, rgb)
    out_arrs, out_names, out_avals = _run(in_maps)
    qi = out_names.index("qout")
    q = np.asarray(out_arrs[qi]).reshape(NCORES, S, L).reshape(N, L)
    return np.ascontiguousarray(q[None].astype(np.float32))


def time_kernel(unaries, rgb, iters=20):
    """Steady-state per-call wall time of the compiled 8-core executable,
    with inputs pre-staged on device."""
    import time as _time
    import jax
    in_maps = _host_prepare(unaries, rgb)
    fn, in_names, out_names, out_avals, zero_outs = _get_runner()
    concat_in = _concat_inputs(in_maps, in_names)

    def once():
        concat_zeros = [np.zeros((NCORES * z.shape[0], *z.shape[1:]), z.dtype)
                        for z in zero_outs]
        outs = fn(*concat_in, *concat_zeros)
        jax.block_until_ready(outs)
        return outs

    once()  # warm
    times = []
    for _ in range(iters):
        t0 = _time.perf_counter()
        once()
        times.append(_time.perf_counter() - t0)
    return min(times), sorted(times)[len(times) // 2]


# revision 53
# speedup vs baseline: 1.7659x; 1.1721x over previous
"""Trainium2 Bass kernel: dense-CRF mean-field layer (96x96 image, 21 labels).

Strategy (8 NeuronCores, row-sharded, K-stationary form):
  * Bilateral kernel K_bl [N,N] is built once on-device (fused feature matmul
    + exp) in bf16 and stays SBUF-resident per core as its [all j, own i]
    slice.
  * The per-iteration bilateral message uses K_bl tiles as the STATIONARY
    matmul operand and streams q chunks [128, 22] as the moving operand:
    out[96 own-pixels, 22] accumulates over 72 j-chunks.  Output is
    pixel-major, so the softmax/combine chain runs directly on [x, y, l]
    tiles with no transposes.
  * Spatial kernel is separable: y-blur is done with t1 (x-blurred q from the
    previous iteration, all-gathered) as the stationary operand per label,
    x-blur per own row after the softmax.  W_SPATIAL/norm folded into the
    host-prepared blur matrices.
  * Per iteration the new q is written straight into the all-gather payload
    (6 partition-shift DMA pieces), together with the x-blurred t1.
"""
import sys
sys.path.insert(0, "/opt/trn_rl_repo")
import os
import numpy as np
import ml_dtypes

H = W = 96
N = H * W                  # 9216
L = 21
LE = L + 1                 # 22 channels (21 labels + norm channel)
ALPHA, BETA, GAMMA = 80.0, 13.0, 3.0
W_SPATIAL, W_BILATERAL = 3.0, 10.0
NUM_ITERATIONS = 5
NCORES = 8
S = N // NCORES            # 1152 rows per core
YPC = H // NCORES          # 12 image rows per core
CH = N // 128              # 72 chunks of 128 rows (global j)
KCOLS = CH * S             # 82944 K_bl sbuf columns (bf16)
QCOLS = CH * LE            # 1584
PAYQ_F32 = 128 * 9 * LE // 2   # 12672 f32 slots holding the bf16 q-part
PAYT = S * LE // 2             # 12672 f32 slots holding the bf16 t1 part
PAY = PAYQ_F32 + PAYT          # 25344
ONESV = 0.1                # q norm-channel value => reciprocal gives 10/norm

# Schraudolph bf16 exp: bits_u16 = trunc(A_SCH * max(x + SH_SCH, 0)),
# bitcast as bf16 ~= exp(x) (max rel err 3.3%, C=5 fitted numerically)
A_SCH = 128.0 / 0.6931471805599453
SH_SCH = (16256.0 - 5.0) / A_SCH

# partition-shift piece groups for the 96->128 repack of q into the payload:
# maps qyb[x0:x0+n, yi, yo, l] -> q128[p0:p0+n, ao, yi, l]  (ai == yi)
# constraint: yo*96 == ao*128 - x0 + p0
QPIECES = ((0, 96, 0, 0, 0),
           (0, 96, 32, 3, 2),
           (0, 32, 96, 1, 0),
           (32, 64, 0, 1, 1),
           (0, 64, 64, 2, 1),
           (64, 32, 0, 2, 2))

LAST_EXEC_NS = None
_CACHE = {}


def _build_bass(sim1=False):
    """Build the kernel. sim1=True builds a single-core variant where the
    AllGather is replaced by 8 local DRAM copies (for TimelineSim analysis)."""
    key = "nc_sim1" if sim1 else "nc"
    if key in _CACHE:
        return _CACHE[key]
    import concourse.bass as bass  # noqa: F401
    from concourse import bacc
    import concourse.mybir as mybir
    import concourse.tile as tile

    f32 = mybir.dt.float32
    bf16 = mybir.dt.bfloat16
    AF = mybir.ActivationFunctionType
    OP = mybir.AluOpType
    AX = mybir.AxisListType

    dbg = bool(int(os.environ.get("CRF_DEBUG", "0"))) and not sim1
    nc = bacc.Bacc("TRN2", target_bir_lowering=False, debug=False,
                   num_devices=1 if sim1 else NCORES)

    featL_d = nc.dram_tensor("featL", [21, N], bf16, kind="ExternalInput")
    featR_d = nc.dram_tensor("featR", [21, S], bf16, kind="ExternalInput")
    uSB_d = nc.dram_tensor("uSB", [W, YPC * L], f32, kind="ExternalInput")
    Ax_d = nc.dram_tensor("Ax", [W, W], bf16, kind="ExternalInput")
    Ay_d = nc.dram_tensor("Ay", [H, YPC], bf16, kind="ExternalInput")
    qsb0_d = nc.dram_tensor("qsb0", [128, QCOLS], bf16, kind="ExternalInput")
    t1f0_d = nc.dram_tensor("t1f0", [H, LE * W], bf16, kind="ExternalInput")
    qout_d = nc.dram_tensor("qout", [S, L], f32, kind="ExternalOutput")
    if dbg:
        dbg_kbl = nc.dram_tensor("dbg_kbl", [128, S], bf16, kind="ExternalOutput")
        dbg_pbl = nc.dram_tensor("dbg_pbl", [W, YPC * LE], f32, kind="ExternalOutput")
        dbg_v = nc.dram_tensor("dbg_v", [W, YPC * L], f32, kind="ExternalOutput")
        dbg_lg = nc.dram_tensor("dbg_lg", [W, YPC * L], f32, kind="ExternalOutput")
        dbg_qy = nc.dram_tensor("dbg_qy", [W, YPC * L], f32, kind="ExternalOutput")
        dbg_t1t = nc.dram_tensor("dbg_t1t", [LE, S], bf16, kind="ExternalOutput")

    # combine groups: rows with y % 4 == yo finish together so the payload
    # piece(s) for that yo can fire while later groups still accumulate
    YGROUPS = [(yo, [yo, yo + 4, yo + 8]) for yo in range(4)]

    with tile.TileContext(nc) as tc:
        with (
            tc.tile_pool(name="const", bufs=1) as constp,
            tc.tile_pool(name="kbl", bufs=1) as kblp,
            tc.tile_pool(name="work", bufs=1) as work,
            tc.tile_pool(name="dram", bufs=2, space="DRAM") as dram,
        ):
            Ax = constp.tile([W, W], bf16)
            Ay = constp.tile([H, YPC], bf16)
            uSB = constp.tile([W, YPC * L], f32)
            Kbl = kblp.tile([128, KCOLS], bf16)
            qsb = work.tile([128, QCOLS], bf16, tag="qsb", bufs=2)
            t1full = work.tile([H, LE * W], bf16, tag="t1full", bufs=2)

            # ---------- precompute K_bl = exp(-||g_i - g_j||^2 / 2) ----------
            # Elementwise exp split across ACT (table exp) and DVE/Pool
            # (Schraudolph bf16-bitcast exp) over 512-col PSUM windows; the
            # 8-deep window ring lets all three engines run concurrently.
            u16 = mybir.dt.uint16
            # A: ACT table-exp from PSUM.  P: DVE shift+clamp PSUM->SBUF f32,
            # then Pool scale+u16-convert SBUF->SBUF (GPSIMD can't touch
            # PSUM).  D: DVE does both steps.
            WPAT = "AADAADAADAADAADA"  # ACT + DVE Schraudolph split
            with (
                tc.tile_pool(name="pre_sb", bufs=2) as pre_sb,
                tc.tile_pool(name="stg", bufs=2) as stgp,
                tc.tile_pool(name="featRp", bufs=1) as featRp,
                tc.tile_pool(name="pre_ps", bufs=8, space="PSUM") as pre_ps,
            ):
                featR = featRp.tile([21, S], bf16)
                nc.sync.dma_start(featR[:], featR_d[:])
                flb, flb_idx = None, -1
                NW = KCOLS // 512
                for wdx in range(NW):
                    if wdx == 1:
                        # late-need loads, queued behind featR + first flb
                        nc.sync.dma_start(qsb[:], qsb0_d[:])
                        nc.sync.dma_start(t1full[:], t1f0_d[:])
                    if wdx == 2:
                        nc.sync.dma_start(Ax[:], Ax_d[:])
                        nc.sync.dma_start(Ay[:], Ay_d[:])
                        nc.sync.dma_start(uSB[:], uSB_d[:])
                    g0 = wdx * 512
                    d2 = pre_ps.tile([128, 512], f32, tag="d2")
                    a = g0
                    while a < g0 + 512:
                        ch = a // S
                        b = min(g0 + 512, (ch + 1) * S)
                        if ch // 8 != flb_idx:
                            flb_idx = ch // 8
                            flb = pre_sb.tile([21, 1024], bf16, tag="fl")
                            nc.sync.dma_start(
                                flb[:],
                                featL_d[:, flb_idx * 1024:(flb_idx + 1) * 1024])
                        nc.tensor.matmul(
                            d2[:, a - g0:b - g0],
                            flb[:, (ch % 8) * 128:(ch % 8 + 1) * 128],
                            featR[:, a - ch * S:b - ch * S],
                            start=True, stop=True)
                        a = b
                    e = WPAT[wdx % len(WPAT)]
                    if e == "A":
                        nc.scalar.activation(Kbl[:, g0:g0 + 512],
                                             d2[:, 0:512], AF.Exp)
                    elif e == "D":
                        nc.vector.tensor_scalar(d2[:, 0:512], d2[:, 0:512],
                                                SH_SCH, 0.0,
                                                op0=OP.add, op1=OP.max)
                        nc.vector.tensor_scalar(
                            Kbl[:, g0:g0 + 512].bitcast(u16),
                            d2[:, 0:512], A_SCH, None, op0=OP.mult)
                    else:
                        stg = stgp.tile([128, 512], f32, tag="stg")
                        nc.vector.tensor_scalar(stg[:], d2[:, 0:512],
                                                SH_SCH, 0.0,
                                                op0=OP.add, op1=OP.max)
                        nc.gpsimd.tensor_scalar(
                            Kbl[:, g0:g0 + 512].bitcast(u16),
                            stg[:], A_SCH, None, op0=OP.mult)

            if dbg:
                nc.sync.dma_start(dbg_kbl.ap(), Kbl[:, 0:S])

            # ---------- mean-field iterations ----------
            psBL_ctx = tc.tile_pool(name="psBL", bufs=1, space="PSUM")
            psBL = psBL_ctx.__enter__()
            psSP_ctx = tc.tile_pool(name="psSP", bufs=1, space="PSUM")
            psSP = psSP_ctx.__enter__()
            psXB_ctx = tc.tile_pool(name="psXB", bufs=2, space="PSUM")
            psXB = psXB_ctx.__enter__()
            psWM_ctx = tc.tile_pool(name="psWM", bufs=1, space="PSUM")
            psWM = psWM_ctx.__enter__()

            def pe_warm_fillers(n, dep_kbl=False):
                """Dummy 512-col matmuls that keep the tensor engine's
                p-state ramp alive across DMA-bound stretches.  With
                dep_kbl, filler k reads a late K_bl window so the stream
                paces itself to the end of the build."""
                wm = psWM.tile([LE, 512], f32, tag="warm")
                for k in range(n):
                    w = (NW - n + k) if dep_kbl else (k % 64)
                    nc.tensor.matmul(wm[:], qsb[:, 0:LE],
                                     Kbl[:, w * 512:(w + 1) * 512],
                                     start=True, stop=True)

            # pe_warm_fillers(12, dep_kbl=True)
            qag_prev = None
            for it in range(NUM_ITERATIONS):
                last = it == NUM_ITERATIONS - 1
                if it > 0:
                    qsb = work.tile([128, QCOLS], bf16, tag="qsb", bufs=2)
                    t1full = work.tile([H, LE * W], bf16, tag="t1full",
                                       bufs=2)
                    for r in range(NCORES):
                        tsrc = (qag_prev[r:r + 1, PAYQ_F32:PAY].bitcast(bf16)
                                .rearrange("a b -> (a b)")
                                .rearrange("(l y x) -> y l x",
                                           l=LE, y=YPC, x=W))
                        tdst = (t1full[r * YPC:(r + 1) * YPC, :]
                                .rearrange("y (l x) -> y l x", l=LE, x=W))
                        eng = (nc.sync, nc.scalar,
                               nc.gpsimd)[r % 3] if r > 1 else nc.sync
                        eng.dma_start(tdst, tsrc)
                    for h in (0, 1):
                        qsrc = (qag_prev[h * 4:(h + 1) * 4, 0:PAYQ_F32]
                                .bitcast(bf16)
                                .rearrange("r (p c) -> p r c", p=128))
                        nc.sync.dma_start(
                            qsb[:].rearrange("p (r c) -> p r c", r=NCORES)
                            [:, h * 4:(h + 1) * 4], qsrc)

                # spatial y-blur: per label, strided t1 slice stationary
                sp = psSP.tile([W, L * YPC], f32, tag="sp", bufs=2)
                for lb in range(L):
                    nc.tensor.matmul(sp[:, lb * YPC:(lb + 1) * YPC],
                                     t1full[:, lb * W:(lb + 1) * W], Ay[:],
                                     start=True, stop=True)
                u3 = uSB[:].rearrange("x (y l) -> x y l", l=L)
                sp3 = sp[:].rearrange("x (l y) -> x y l", l=L)
                pbl = psBL.tile([W, YPC * LE], f32, tag="pbl", bufs=2)

                lg = work.tile([W, YPC * L], f32, tag="lg", bufs=2)
                lg3 = lg[:].rearrange("x (y l) -> x y l", l=L)
                qy = work.tile([W, YPC * L], f32, tag="qy", bufs=2)
                qy3 = qy[:].rearrange("x (y l) -> x y l", l=L)
                ssum = work.tile([W, YPC], f32, tag="ssum", bufs=2)
                rec = work.tile([W, YPC], f32, tag="rec", bufs=2)
                if last:
                    qyf = work.tile([W, YPC * LE], f32, tag="qyf")
                    qyf3 = qyf[:].rearrange("x (y l) -> x y l", l=LE)
                else:
                    # per-group q tiles keep the payload-piece DMA deps
                    # narrow (whole-tile tracking would defer every piece
                    # to the last group's softmax)
                    qybs = [work.tile([W, 3 * LE], bf16, tag=f"qyb{g}",
                                      name=f"qyb{g}", bufs=2)
                            for g in range(4)]
                    t1X = work.tile([LE, S], bf16, tag="t1X", bufs=2)
                    pl = dram.tile([1, PAY], f32, tag="pl")
                    plq = (pl[0:1, 0:PAYQ_F32].bitcast(bf16)
                           .rearrange("a (p ai ao l) -> (a p) ao ai l",
                                      p=128, ai=3, ao=3))
                pbl3 = pbl[:].rearrange("x (y l) -> x y l", l=LE)

                def emit_xblur(yo):
                    # x-blur rows y%4==yo (new q stationary, Ax moving)
                    gsl = slice(yo, yo + 9, 4)
                    xb = psXB.tile([LE, 3 * W], f32, tag="xb")
                    for k, r in enumerate((yo, yo + 4, yo + 8)):
                        nc.tensor.matmul(xb[:, k * W:(k + 1) * W],
                                         qybs[yo][:, k * LE:(k + 1) * LE],
                                         Ax[:], start=True, stop=True)
                    nc.scalar.copy(
                        t1X[:].rearrange("l (y x) -> l y x", x=W)[:, gsl],
                        xb[:].rearrange("l (k x) -> l k x", x=W))

                for (yo, ys) in YGROUPS:
                    for r in ys:
                        for ch in range(CH):
                            nc.tensor.matmul(
                                pbl[:, r * LE:(r + 1) * LE],
                                Kbl[:, ch * S + r * W: ch * S + (r + 1) * W],
                                qsb[:, ch * LE:(ch + 1) * LE],
                                start=(ch == 0), stop=(ch == CH - 1))
                    # previous group's x-blur: emitted here so the PE never
                    # waits on the previous group's softmax chain
                    if yo > 0 and not last:
                        emit_xblur(yo - 1)
                    # combine + softmax for this group's 3 rows
                    gsl = slice(yo, yo + 9, 4)  # rows yo, yo+4, yo+8
                    nc.vector.reciprocal(rec[:, gsl][:, :, None],
                                         pbl3[:, gsl, L:LE])
                    nc.vector.tensor_tensor(
                        lg3[:, gsl], pbl3[:, gsl, 0:L],
                        rec[:, gsl][:, :, None].to_broadcast([W, 3, L]),
                        OP.mult)
                    nc.vector.tensor_tensor(lg3[:, gsl], lg3[:, gsl],
                                            u3[:, gsl], OP.add)
                    nc.vector.tensor_tensor(lg3[:, gsl], lg3[:, gsl],
                                            sp3[:, gsl], OP.add)
                    nc.scalar.activation(qy3[:, gsl], lg3[:, gsl], AF.Exp)
                    nc.vector.reduce_sum(ssum[:, gsl], qy3[:, gsl], axis=AX.X)
                    nc.vector.reciprocal(ssum[:, gsl], ssum[:, gsl])
                    qt = (qyf3[:, gsl] if last
                          else qybs[yo][:].rearrange("x (k l) -> x k l", l=LE))
                    nc.vector.tensor_tensor(
                        qt[:, :, 0:L], qy3[:, gsl],
                        ssum[:, gsl][:, :, None].to_broadcast([W, 3, L]),
                        OP.mult)
                    if last:
                        continue
                    if it < 2:
                        nc.vector.memset(qt[:, :, L:LE], ONESV)
                    # payload piece(s) for this yo straight into DRAM, on
                    # the otherwise-idle SWDGE queue
                    for (x0, n, p0, pyo, ao) in QPIECES:
                        if pyo == yo:
                            nc.gpsimd.dma_start(
                                plq[p0:p0 + n, ao],
                                qybs[yo][x0:x0 + n, :]
                                .rearrange("x (k l) -> x k l", l=LE))
                if not last:
                    emit_xblur(3)

                if dbg and it == 0:
                    nc.sync.dma_start(dbg_pbl.ap(), pbl[:])
                    nc.sync.dma_start(dbg_lg.ap(), lg[:])
                if dbg and it == 1:
                    nc.sync.dma_start(dbg_qy.ap(), qy[:])
                    nc.sync.dma_start(dbg_t1t.ap(), t1X[:])

                if last:
                    nc.sync.dma_start(
                        qout_d.ap().rearrange("(y x) l -> x y l", x=W),
                        qyf3[:, :, 0:L])
                    continue

                # t1 payload part (layout (l, y, x) per core)
                nc.scalar.dma_start(
                    pl[0:1, PAYQ_F32:PAY].bitcast(bf16)
                      .rearrange("a (l c) -> (a l) c", l=LE),
                    t1X[:])
                # AllGather (sim1: two broadcast DRAM copies, same data
                # volume as 8 per-peer copies)
                qag = dram.tile([NCORES, PAY], f32, tag="qag")
                if sim1:
                    nc.sync.dma_start(qag[0:4, :],
                                      pl[:].to_broadcast([4, PAY]))
                    nc.sync.dma_start(qag[4:8, :],
                                      pl[:].to_broadcast([4, PAY]))
                else:
                    nc.gpsimd.collective_compute(
                        "AllGather", OP.bypass,
                        replica_groups=[list(range(NCORES))],
                        ins=[pl.opt()], outs=[qag.opt()])
                qag_prev = qag
            psWM_ctx.__exit__(None, None, None)
            psXB_ctx.__exit__(None, None, None)
            psSP_ctx.__exit__(None, None, None)
            psBL_ctx.__exit__(None, None, None)

    nc.compile()
    _CACHE[key] = nc
    return nc


def _host_prepare(unaries, rgb):
    u = np.asarray(unaries, np.float32).reshape(N, L)
    c = np.asarray(rgb, np.float32).reshape(N, 3)

    ys, xs = np.meshgrid(np.arange(H, dtype=np.float64),
                         np.arange(W, dtype=np.float64), indexing="ij")
    pos = np.stack([ys.ravel(), xs.ravel()], -1)            # [N, 2]
    g = np.concatenate([c.astype(np.float64) / BETA, pos / ALPHA], 1)
    g = g - g.mean(0, keepdims=True)
    sq = (g * g).sum(1)
    ones = np.ones(N, np.float64)
    L7 = np.concatenate([g.T, ones[None], (-0.5 * sq)[None]], 0)  # [7, N] j
    R7 = np.concatenate([g.T, (-0.5 * sq)[None], ones[None]], 0)  # [7, N] i
    bfd = ml_dtypes.bfloat16
    Lhi = L7.astype(bfd)
    Llo = (L7 - Lhi.astype(np.float64)).astype(bfd)
    Rhi = R7.astype(bfd)
    Rlo = (R7 - Rhi.astype(np.float64)).astype(bfd)
    # dot = Lhi.Rhi + Lhi.Rlo + Llo.Rhi  (Llo.Rlo dropped, ~1e-3)
    featL = np.ascontiguousarray(np.concatenate([Lhi, Lhi, Llo], 0))  # [21,N]
    featR = np.ascontiguousarray(np.concatenate([Rhi, Rlo, Rhi], 0))  # [21,N]

    d = np.arange(W, dtype=np.float64)
    A = np.exp(-(d[:, None] - d[None, :]) ** 2 / (2.0 * GAMMA * GAMMA))
    nvec = A.sum(0)
    Ax = np.ascontiguousarray((A / nvec[None, :]).astype(ml_dtypes.bfloat16))

    um = u.max(1, keepdims=True)
    e = np.exp(u - um)
    q0 = e / e.sum(1, keepdims=True)
    q0e = np.concatenate([q0, np.full((N, 1), ONESV, np.float32)], 1)  # [N,22]
    qsb0 = np.ascontiguousarray(
        q0e.reshape(CH, 128, LE).transpose(1, 0, 2).reshape(128, QCOLS)
    ).astype(ml_dtypes.bfloat16)

    q3 = q0e.reshape(H, W, LE).astype(np.float64)
    t1 = np.einsum("Xx,yXl->ylx", A / nvec[None, :], q3)      # [96, 22, 96]
    t1f0 = np.ascontiguousarray(t1.reshape(H, LE * W).astype(ml_dtypes.bfloat16))

    in_maps = []
    for core in range(NCORES):
        rows = slice(core * S, (core + 1) * S)
        uSB_c = np.ascontiguousarray(
            u[rows].reshape(YPC, W, L).transpose(1, 0, 2).reshape(W, YPC * L))
        yc = slice(core * YPC, (core + 1) * YPC)
        Ay_c = np.ascontiguousarray(
            (A[:, yc] * (W_SPATIAL / nvec[yc])[None, :]).astype(ml_dtypes.bfloat16))
        in_maps.append({
            "featL": featL,
            "featR": np.ascontiguousarray(featR[:, rows]),
            "uSB": uSB_c,
            "Ax": Ax,
            "Ay": Ay_c,
            "qsb0": qsb0,
            "t1f0": t1f0,
        })
    return in_maps


def _get_runner():
    """Compile once; return (fn, in_names, out_names) where fn maps
    concatenated global numpy inputs -> list of per-core output dicts."""
    if "runner" in _CACHE:
        return _CACHE["runner"]
    import jax
    from jax.sharding import Mesh, PartitionSpec
    from jax.experimental.shard_map import shard_map
    import concourse.mybir as mybir
    from concourse import bass2jax

    nc = _build_bass()
    bass2jax.install_neuronx_cc_hook()

    partition_name = (nc.partition_id_tensor.name
                      if nc.partition_id_tensor else None)
    in_names, out_names, out_avals, zero_outs = [], [], [], []
    for alloc in nc.m.functions[0].allocations:
        if not isinstance(alloc, mybir.MemoryLocationSet):
            continue
        name = alloc.memorylocations[0].name
        if alloc.kind == "ExternalInput":
            if name != partition_name:
                in_names.append(name)
        elif alloc.kind == "ExternalOutput":
            shape = tuple(alloc.tensor_shape)
            dtype = mybir.dt.np(alloc.dtype)
            out_names.append(name)
            out_avals.append(jax.core.ShapedArray(shape, dtype))
            zero_outs.append(np.zeros(shape, dtype))
    n_params = len(in_names)
    all_in_names = list(in_names) + list(out_names)
    if partition_name is not None:
        all_in_names.append(partition_name)

    def _body(*args):
        operands = list(args)
        if partition_name is not None:
            operands.append(bass2jax.partition_id_tensor())
        outs = bass2jax._bass_exec_p.bind(
            *operands,
            out_avals=tuple(out_avals),
            in_names=tuple(all_in_names),
            out_names=tuple(out_names),
            lowering_input_output_aliases=(),
            sim_require_finite=False,
            sim_require_nnan=False,
            nc=nc,
        )
        return tuple(outs)

    devices = jax.devices()[:NCORES]
    mesh = Mesh(np.asarray(devices), ("core",))
    n_outs = len(out_names)
    in_specs = (PartitionSpec("core"),) * (n_params + n_outs)
    out_specs = (PartitionSpec("core"),) * n_outs
    donate = tuple(range(n_params, n_params + n_outs))
    fn = jax.jit(
        shard_map(_body, mesh=mesh, in_specs=in_specs, out_specs=out_specs,
                  check_rep=False),
        donate_argnums=donate, keep_unused=True)
    _CACHE["runner"] = (fn, in_names, out_names, out_avals, zero_outs)
    return _CACHE["runner"]


def _concat_inputs(in_maps, in_names):
    return [np.concatenate([np.asarray(in_maps[c][nm]) for c in range(NCORES)],
                           axis=0) for nm in in_names]


def _run(in_maps):
    fn, in_names, out_names, out_avals, zero_outs = _get_runner()
    concat_in = _concat_inputs(in_maps, in_names)
    concat_zeros = [np.zeros((NCORES * z.shape[0], *z.shape[1:]), z.dtype)
                    for z in zero_outs]
    out_arrs = fn(*concat_in, *concat_zeros)
    return out_arrs, out_names, out_avals


def kernel(unaries, rgb):
    in_maps = _host_prepare(unaries, rgb)
    out_arrs, out_names, out_avals = _run(in_maps)
    qi = out_names.index("qout")
    q = np.asarray(out_arrs[qi]).reshape(NCORES, S, L).reshape(N, L)
    return np.ascontiguousarray(q[None].astype(np.float32))


def time_kernel(unaries, rgb, iters=20):
    """Steady-state per-call wall time of the compiled 8-core executable,
    with inputs pre-staged on device."""
    import time as _time
    import jax
    in_maps = _host_prepare(unaries, rgb)
    fn, in_names, out_names, out_avals, zero_outs = _get_runner()
    concat_in = _concat_inputs(in_maps, in_names)

    def once():
        concat_zeros = [np.zeros((NCORES * z.shape[0], *z.shape[1:]), z.dtype)
                        for z in zero_outs]
        outs = fn(*concat_in, *concat_zeros)
        jax.block_until_ready(outs)
        return outs

    once()  # warm
    times = []
    for _ in range(iters):
        t0 = _time.perf_counter()
        once()
        times.append(_time.perf_counter() - t0)
    return min(times), sorted(times)[len(times) // 2]


# revision 56
# speedup vs baseline: 1.8245x; 1.0332x over previous
"""Trainium2 Bass kernel: dense-CRF mean-field layer (96x96 image, 21 labels).

Strategy (8 NeuronCores, row-sharded, K-stationary form):
  * Bilateral kernel K_bl [N,N] is built once on-device (fused feature matmul
    + exp) in bf16 and stays SBUF-resident per core as its [all j, own i]
    slice.
  * The per-iteration bilateral message uses K_bl tiles as the STATIONARY
    matmul operand and streams q chunks [128, 22] as the moving operand:
    out[96 own-pixels, 22] accumulates over 72 j-chunks.  Output is
    pixel-major, so the softmax/combine chain runs directly on [x, y, l]
    tiles with no transposes.
  * Spatial kernel is separable: y-blur is done with t1 (x-blurred q from the
    previous iteration, all-gathered) as the stationary operand per label,
    x-blur per own row after the softmax.  W_SPATIAL/norm folded into the
    host-prepared blur matrices.
  * Per iteration the new q is written straight into the all-gather payload
    (6 partition-shift DMA pieces), together with the x-blurred t1.
"""
import sys
sys.path.insert(0, "/opt/trn_rl_repo")
import os
import numpy as np
import ml_dtypes

H = W = 96
N = H * W                  # 9216
L = 21
LE = L + 1                 # 22 channels (21 labels + norm channel)
ALPHA, BETA, GAMMA = 80.0, 13.0, 3.0
W_SPATIAL, W_BILATERAL = 3.0, 10.0
NUM_ITERATIONS = 5
NCORES = 8
S = N // NCORES            # 1152 rows per core
YPC = H // NCORES          # 12 image rows per core
CH = N // 128              # 72 chunks of 128 rows (global j)
KCOLS = CH * S             # 82944 K_bl sbuf columns (bf16)
QCOLS = CH * LE            # 1584
PAYQ_F32 = 128 * 9 * LE // 2   # 12672 f32 slots holding the bf16 q-part
PAYT = S * LE // 2             # 12672 f32 slots holding the bf16 t1 part
PAY = PAYQ_F32 + PAYT          # 25344
ONESV = 0.1                # q norm-channel value => reciprocal gives 10/norm

# Schraudolph bf16 exp: bits_u16 = trunc(A_SCH * max(x + SH_SCH, 0)),
# bitcast as bf16 ~= exp(x) (max rel err 3.3%, C=5 fitted numerically)
A_SCH = 128.0 / 0.6931471805599453
SH_SCH = (16256.0 - 5.0) / A_SCH

# partition-shift piece groups for the 96->128 repack of q into the payload:
# maps qyb[x0:x0+n, yi, yo, l] -> q128[p0:p0+n, ao, yi, l]  (ai == yi)
# constraint: yo*96 == ao*128 - x0 + p0
QPIECES = ((0, 96, 0, 0, 0),
           (0, 96, 32, 3, 2),
           (0, 32, 96, 1, 0),
           (32, 64, 0, 1, 1),
           (0, 64, 64, 2, 1),
           (64, 32, 0, 2, 2))

LAST_EXEC_NS = None
_CACHE = {}


def _build_bass(sim1=False):
    """Build the kernel. sim1=True builds a single-core variant where the
    AllGather is replaced by 8 local DRAM copies (for TimelineSim analysis)."""
    key = "nc_sim1" if sim1 else "nc"
    if key in _CACHE:
        return _CACHE[key]
    import concourse.bass as bass  # noqa: F401
    from concourse import bacc
    import concourse.mybir as mybir
    import concourse.tile as tile

    f32 = mybir.dt.float32
    bf16 = mybir.dt.bfloat16
    AF = mybir.ActivationFunctionType
    OP = mybir.AluOpType
    AX = mybir.AxisListType

    dbg = bool(int(os.environ.get("CRF_DEBUG", "0"))) and not sim1
    nc = bacc.Bacc("TRN2", target_bir_lowering=False, debug=False,
                   num_devices=1 if sim1 else NCORES)

    featL_d = nc.dram_tensor("featL", [21, N], bf16, kind="ExternalInput")
    featR_d = nc.dram_tensor("featR", [21, S], bf16, kind="ExternalInput")
    uSB_d = nc.dram_tensor("uSB", [W, YPC * L], f32, kind="ExternalInput")
    Ax_d = nc.dram_tensor("Ax", [W, W], bf16, kind="ExternalInput")
    Ay_d = nc.dram_tensor("Ay", [H, YPC], bf16, kind="ExternalInput")
    qsb0_d = nc.dram_tensor("qsb0", [128, QCOLS], bf16, kind="ExternalInput")
    t1f0_d = nc.dram_tensor("t1f0", [H, LE * W], bf16, kind="ExternalInput")
    qout_d = nc.dram_tensor("qout", [S, L], f32, kind="ExternalOutput")
    if dbg:
        dbg_kbl = nc.dram_tensor("dbg_kbl", [128, S], bf16, kind="ExternalOutput")
        dbg_pbl = nc.dram_tensor("dbg_pbl", [W, YPC * LE], f32, kind="ExternalOutput")
        dbg_v = nc.dram_tensor("dbg_v", [W, YPC * L], f32, kind="ExternalOutput")
        dbg_lg = nc.dram_tensor("dbg_lg", [W, YPC * L], f32, kind="ExternalOutput")
        dbg_qy = nc.dram_tensor("dbg_qy", [W, YPC * L], f32, kind="ExternalOutput")
        dbg_t1t = nc.dram_tensor("dbg_t1t", [LE, S], bf16, kind="ExternalOutput")

    # combine groups: rows with y % 4 == yo finish together so the payload
    # piece(s) for that yo can fire while later groups still accumulate
    YGROUPS = [(yo, [yo, yo + 4, yo + 8]) for yo in range(4)]

    with tile.TileContext(nc) as tc:
        with (
            tc.tile_pool(name="const", bufs=1) as constp,
            tc.tile_pool(name="kbl", bufs=1) as kblp,
            tc.tile_pool(name="work", bufs=1) as work,
            tc.tile_pool(name="dram", bufs=2, space="DRAM") as dram,
        ):
            Ax = constp.tile([W, W], bf16)
            Ay = constp.tile([H, YPC], bf16)
            uSB = constp.tile([W, YPC * L], f32)
            Kbl = kblp.tile([128, KCOLS], bf16)
            qsb = work.tile([128, QCOLS], bf16, tag="qsb", bufs=2)
            t1full = work.tile([H, LE * W], bf16, tag="t1full", bufs=2)

            # ---------- precompute K_bl = exp(-||g_i - g_j||^2 / 2) ----------
            # Elementwise exp split across ACT (table exp) and DVE/Pool
            # (Schraudolph bf16-bitcast exp) over 512-col PSUM windows; the
            # 8-deep window ring lets all three engines run concurrently.
            u16 = mybir.dt.uint16
            # A: ACT table-exp from PSUM.  P: DVE shift+clamp PSUM->SBUF f32,
            # then Pool scale+u16-convert SBUF->SBUF (GPSIMD can't touch
            # PSUM).  D: DVE does both steps.
            WPAT = "AADAADAADAADA"  # ACT + DVE Schraudolph split (per-1024)
            with (
                tc.tile_pool(name="pre_sb", bufs=2) as pre_sb,
                tc.tile_pool(name="stg", bufs=2) as stgp,
                tc.tile_pool(name="featRp", bufs=1) as featRp,
                tc.tile_pool(name="pre_ps", bufs=4, space="PSUM") as pre_ps,
            ):
                featR = featRp.tile([21, S], bf16)
                nc.sync.dma_start(featR[:], featR_d[:])
                flb, flb_idx = None, -1
                WIN = 1024
                NW = KCOLS // WIN
                for wdx in range(NW):
                    if wdx == 1:
                        # late-need loads, queued behind featR + first flb
                        nc.sync.dma_start(qsb[:], qsb0_d[:])
                        nc.sync.dma_start(t1full[:], t1f0_d[:])
                    if wdx == 2:
                        nc.sync.dma_start(Ax[:], Ax_d[:])
                        nc.sync.dma_start(Ay[:], Ay_d[:])
                        nc.sync.dma_start(uSB[:], uSB_d[:])
                    g0 = wdx * WIN
                    d2 = pre_ps.tile([128, WIN], f32, tag="d2")
                    cuts = sorted({g0, g0 + WIN}
                                  | set(range((g0 // 512 + 1) * 512,
                                              g0 + WIN, 512))
                                  | set(range((g0 // S + 1) * S,
                                              g0 + WIN, S)))
                    for a, b in zip(cuts[:-1], cuts[1:]):
                        ch = a // S
                        if ch // 8 != flb_idx:
                            flb_idx = ch // 8
                            flb = pre_sb.tile([21, 1024], bf16, tag="fl")
                            nc.sync.dma_start(
                                flb[:],
                                featL_d[:, flb_idx * 1024:(flb_idx + 1) * 1024])
                        nc.tensor.matmul(
                            d2[:, a - g0:b - g0],
                            flb[:, (ch % 8) * 128:(ch % 8 + 1) * 128],
                            featR[:, a - ch * S:b - ch * S],
                            start=True, stop=True)
                    e = WPAT[wdx % len(WPAT)]
                    if e == "A":
                        nc.scalar.activation(Kbl[:, g0:g0 + WIN],
                                             d2[:, 0:WIN], AF.Exp)
                    else:
                        nc.vector.tensor_scalar(d2[:, 0:WIN], d2[:, 0:WIN],
                                                SH_SCH, 0.0,
                                                op0=OP.add, op1=OP.max)
                        nc.vector.tensor_scalar(
                            Kbl[:, g0:g0 + WIN].bitcast(u16),
                            d2[:, 0:WIN], A_SCH, None, op0=OP.mult)

            if dbg:
                nc.sync.dma_start(dbg_kbl.ap(), Kbl[:, 0:S])

            # ---------- mean-field iterations ----------
            psBL_ctx = tc.tile_pool(name="psBL", bufs=1, space="PSUM")
            psBL = psBL_ctx.__enter__()
            psSP_ctx = tc.tile_pool(name="psSP", bufs=1, space="PSUM")
            psSP = psSP_ctx.__enter__()
            psXB_ctx = tc.tile_pool(name="psXB", bufs=2, space="PSUM")
            psXB = psXB_ctx.__enter__()
            psWM_ctx = tc.tile_pool(name="psWM", bufs=1, space="PSUM")
            psWM = psWM_ctx.__enter__()

            def pe_warm_fillers(n, dep_kbl=False):
                """Dummy 512-col matmuls that keep the tensor engine's
                p-state ramp alive across DMA-bound stretches.  With
                dep_kbl, filler k reads a late K_bl window so the stream
                paces itself to the end of the build."""
                wm = psWM.tile([LE, 512], f32, tag="warm")
                for k in range(n):
                    w = (NW - n + k) if dep_kbl else (k % 64)
                    nc.tensor.matmul(wm[:], qsb[:, 0:LE],
                                     Kbl[:, w * 512:(w + 1) * 512],
                                     start=True, stop=True)

            # pe_warm_fillers(12, dep_kbl=True)
            qag_prev = None
            for it in range(NUM_ITERATIONS):
                last = it == NUM_ITERATIONS - 1
                if it > 0:
                    qsb = work.tile([128, QCOLS], bf16, tag="qsb", bufs=2)
                    t1full = work.tile([H, LE * W], bf16, tag="t1full",
                                       bufs=2)
                    for r in range(NCORES):
                        tsrc = (qag_prev[r:r + 1, PAYQ_F32:PAY].bitcast(bf16)
                                .rearrange("a b -> (a b)")
                                .rearrange("(l y x) -> y l x",
                                           l=LE, y=YPC, x=W))
                        tdst = (t1full[r * YPC:(r + 1) * YPC, :]
                                .rearrange("y (l x) -> y l x", l=LE, x=W))
                        eng = (nc.sync if r < 2 else
                               (nc.scalar if r % 2 else nc.gpsimd))
                        eng.dma_start(tdst, tsrc)
                    for h in (0, 1):
                        qsrc = (qag_prev[h * 4:(h + 1) * 4, 0:PAYQ_F32]
                                .bitcast(bf16)
                                .rearrange("r (p c) -> p r c", p=128))
                        nc.sync.dma_start(
                            qsb[:].rearrange("p (r c) -> p r c", r=NCORES)
                            [:, h * 4:(h + 1) * 4], qsrc)

                # spatial y-blur: per label, strided t1 slice stationary
                sp = psSP.tile([W, L * YPC], f32, tag="sp", bufs=2)
                for lb in range(L):
                    nc.tensor.matmul(sp[:, lb * YPC:(lb + 1) * YPC],
                                     t1full[:, lb * W:(lb + 1) * W], Ay[:],
                                     start=True, stop=True)
                u3 = uSB[:].rearrange("x (y l) -> x y l", l=L)
                sp3 = sp[:].rearrange("x (l y) -> x y l", l=L)
                pbl = psBL.tile([W, YPC * LE], f32, tag="pbl", bufs=2)

                lg = work.tile([W, YPC * L], f32, tag="lg", bufs=2)
                lg3 = lg[:].rearrange("x (y l) -> x y l", l=L)
                qy = work.tile([W, YPC * L], f32, tag="qy", bufs=2)
                qy3 = qy[:].rearrange("x (y l) -> x y l", l=L)
                ssum = work.tile([W, YPC], f32, tag="ssum", bufs=2)
                rec = work.tile([W, YPC], f32, tag="rec", bufs=2)
                if last:
                    qyf = work.tile([W, YPC * LE], f32, tag="qyf")
                    qyf3 = qyf[:].rearrange("x (y l) -> x y l", l=LE)
                else:
                    # per-group q tiles keep the payload-piece DMA deps
                    # narrow (whole-tile tracking would defer every piece
                    # to the last group's softmax)
                    qybs = [work.tile([W, 3 * LE], bf16, tag=f"qyb{g}",
                                      name=f"qyb{g}", bufs=2)
                            for g in range(4)]
                    t1X = work.tile([LE, S], bf16, tag="t1X", bufs=2)
                    pl = dram.tile([1, PAY], f32, tag="pl")
                    plq = (pl[0:1, 0:PAYQ_F32].bitcast(bf16)
                           .rearrange("a (p ai ao l) -> (a p) ao ai l",
                                      p=128, ai=3, ao=3))
                pbl3 = pbl[:].rearrange("x (y l) -> x y l", l=LE)

                def emit_xblur(yo):
                    # x-blur rows y%4==yo (new q stationary, Ax moving)
                    gsl = slice(yo, yo + 9, 4)
                    xb = psXB.tile([LE, 3 * W], f32, tag="xb")
                    for k, r in enumerate((yo, yo + 4, yo + 8)):
                        nc.tensor.matmul(xb[:, k * W:(k + 1) * W],
                                         qybs[yo][:, k * LE:(k + 1) * LE],
                                         Ax[:], start=True, stop=True)
                    nc.scalar.copy(
                        t1X[:].rearrange("l (y x) -> l y x", x=W)[:, gsl],
                        xb[:].rearrange("l (k x) -> l k x", x=W))

                for (yo, ys) in YGROUPS:
                    for r in ys:
                        for ch in range(CH):
                            nc.tensor.matmul(
                                pbl[:, r * LE:(r + 1) * LE],
                                Kbl[:, ch * S + r * W: ch * S + (r + 1) * W],
                                qsb[:, ch * LE:(ch + 1) * LE],
                                start=(ch == 0), stop=(ch == CH - 1))
                    # previous group's x-blur: emitted here so the PE never
                    # waits on the previous group's softmax chain
                    if yo > 0 and not last:
                        emit_xblur(yo - 1)
                    # combine + softmax for this group's 3 rows
                    gsl = slice(yo, yo + 9, 4)  # rows yo, yo+4, yo+8
                    nc.vector.reciprocal(rec[:, gsl][:, :, None],
                                         pbl3[:, gsl, L:LE])
                    nc.vector.tensor_tensor(
                        lg3[:, gsl], pbl3[:, gsl, 0:L],
                        rec[:, gsl][:, :, None].to_broadcast([W, 3, L]),
                        OP.mult)
                    nc.vector.tensor_tensor(lg3[:, gsl], lg3[:, gsl],
                                            u3[:, gsl], OP.add)
                    nc.vector.tensor_tensor(lg3[:, gsl], lg3[:, gsl],
                                            sp3[:, gsl], OP.add)
                    nc.scalar.activation(qy3[:, gsl], lg3[:, gsl], AF.Exp)
                    nc.vector.reduce_sum(ssum[:, gsl], qy3[:, gsl], axis=AX.X)
                    nc.vector.reciprocal(ssum[:, gsl], ssum[:, gsl])
                    qt = (qyf3[:, gsl] if last
                          else qybs[yo][:].rearrange("x (k l) -> x k l", l=LE))
                    nc.vector.tensor_tensor(
                        qt[:, :, 0:L], qy3[:, gsl],
                        ssum[:, gsl][:, :, None].to_broadcast([W, 3, L]),
                        OP.mult)
                    if last:
                        continue
                    if it < 2:
                        nc.vector.memset(qt[:, :, L:LE], ONESV)
                    # payload piece(s) for this yo straight into DRAM;
                    # the last group's piece goes on SP so it lands just
                    # before the q-section gather on the same queue
                    for (x0, n, p0, pyo, ao) in QPIECES:
                        if pyo == yo:
                            eng = nc.sync if yo == 3 else nc.gpsimd
                            eng.dma_start(
                                plq[p0:p0 + n, ao],
                                qybs[yo][x0:x0 + n, :]
                                .rearrange("x (k l) -> x k l", l=LE))
                if not last:
                    emit_xblur(3)

                if dbg and it == 0:
                    nc.sync.dma_start(dbg_pbl.ap(), pbl[:])
                    nc.sync.dma_start(dbg_lg.ap(), lg[:])
                if dbg and it == 1:
                    nc.sync.dma_start(dbg_qy.ap(), qy[:])
                    nc.sync.dma_start(dbg_t1t.ap(), t1X[:])

                if last:
                    nc.sync.dma_start(
                        qout_d.ap().rearrange("(y x) l -> x y l", x=W),
                        qyf3[:, :, 0:L])
                    continue

                # t1 payload part (layout (l, y, x) per core)
                nc.scalar.dma_start(
                    pl[0:1, PAYQ_F32:PAY].bitcast(bf16)
                      .rearrange("a (l c) -> (a l) c", l=LE),
                    t1X[:])
                # AllGather (sim1: two broadcast DRAM copies, same data
                # volume as 8 per-peer copies)
                qag = dram.tile([NCORES, PAY], f32, tag="qag")
                if sim1:
                    nc.sync.dma_start(qag[0:4, :],
                                      pl[:].to_broadcast([4, PAY]))
                    nc.sync.dma_start(qag[4:8, :],
                                      pl[:].to_broadcast([4, PAY]))
                else:
                    nc.gpsimd.collective_compute(
                        "AllGather", OP.bypass,
                        replica_groups=[list(range(NCORES))],
                        ins=[pl.opt()], outs=[qag.opt()])
                qag_prev = qag
            psWM_ctx.__exit__(None, None, None)
            psXB_ctx.__exit__(None, None, None)
            psSP_ctx.__exit__(None, None, None)
            psBL_ctx.__exit__(None, None, None)

    nc.compile()
    _CACHE[key] = nc
    return nc


def _host_prepare(unaries, rgb):
    u = np.asarray(unaries, np.float32).reshape(N, L)
    c = np.asarray(rgb, np.float32).reshape(N, 3)

    ys, xs = np.meshgrid(np.arange(H, dtype=np.float64),
                         np.arange(W, dtype=np.float64), indexing="ij")
    pos = np.stack([ys.ravel(), xs.ravel()], -1)            # [N, 2]
    g = np.concatenate([c.astype(np.float64) / BETA, pos / ALPHA], 1)
    g = g - g.mean(0, keepdims=True)
    sq = (g * g).sum(1)
    ones = np.ones(N, np.float64)
    L7 = np.concatenate([g.T, ones[None], (-0.5 * sq)[None]], 0)  # [7, N] j
    R7 = np.concatenate([g.T, (-0.5 * sq)[None], ones[None]], 0)  # [7, N] i
    bfd = ml_dtypes.bfloat16
    Lhi = L7.astype(bfd)
    Llo = (L7 - Lhi.astype(np.float64)).astype(bfd)
    Rhi = R7.astype(bfd)
    Rlo = (R7 - Rhi.astype(np.float64)).astype(bfd)
    # dot = Lhi.Rhi + Lhi.Rlo + Llo.Rhi  (Llo.Rlo dropped, ~1e-3)
    featL = np.ascontiguousarray(np.concatenate([Lhi, Lhi, Llo], 0))  # [21,N]
    featR = np.ascontiguousarray(np.concatenate([Rhi, Rlo, Rhi], 0))  # [21,N]

    d = np.arange(W, dtype=np.float64)
    A = np.exp(-(d[:, None] - d[None, :]) ** 2 / (2.0 * GAMMA * GAMMA))
    nvec = A.sum(0)
    Ax = np.ascontiguousarray((A / nvec[None, :]).astype(ml_dtypes.bfloat16))

    um = u.max(1, keepdims=True)
    e = np.exp(u - um)
    q0 = e / e.sum(1, keepdims=True)
    q0e = np.concatenate([q0, np.full((N, 1), ONESV, np.float32)], 1)  # [N,22]
    qsb0 = np.ascontiguousarray(
        q0e.reshape(CH, 128, LE).transpose(1, 0, 2).reshape(128, QCOLS)
    ).astype(ml_dtypes.bfloat16)

    q3 = q0e.reshape(H, W, LE).astype(np.float64)
    t1 = np.einsum("Xx,yXl->ylx", A / nvec[None, :], q3)      # [96, 22, 96]
    t1f0 = np.ascontiguousarray(t1.reshape(H, LE * W).astype(ml_dtypes.bfloat16))

    in_maps = []
    for core in range(NCORES):
        rows = slice(core * S, (core + 1) * S)
        uSB_c = np.ascontiguousarray(
            u[rows].reshape(YPC, W, L).transpose(1, 0, 2).reshape(W, YPC * L))
        yc = slice(core * YPC, (core + 1) * YPC)
        Ay_c = np.ascontiguousarray(
            (A[:, yc] * (W_SPATIAL / nvec[yc])[None, :]).astype(ml_dtypes.bfloat16))
        in_maps.append({
            "featL": featL,
            "featR": np.ascontiguousarray(featR[:, rows]),
            "uSB": uSB_c,
            "Ax": Ax,
            "Ay": Ay_c,
            "qsb0": qsb0,
            "t1f0": t1f0,
        })
    return in_maps


def _get_runner():
    """Compile once; return (fn, in_names, out_names) where fn maps
    concatenated global numpy inputs -> list of per-core output dicts."""
    if "runner" in _CACHE:
        return _CACHE["runner"]
    import jax
    from jax.sharding import Mesh, PartitionSpec
    from jax.experimental.shard_map import shard_map
    import concourse.mybir as mybir
    from concourse import bass2jax

    nc = _build_bass()
    bass2jax.install_neuronx_cc_hook()

    partition_name = (nc.partition_id_tensor.name
                      if nc.partition_id_tensor else None)
    in_names, out_names, out_avals, zero_outs = [], [], [], []
    for alloc in nc.m.functions[0].allocations:
        if not isinstance(alloc, mybir.MemoryLocationSet):
            continue
        name = alloc.memorylocations[0].name
        if alloc.kind == "ExternalInput":
            if name != partition_name:
                in_names.append(name)
        elif alloc.kind == "ExternalOutput":
            shape = tuple(alloc.tensor_shape)
            dtype = mybir.dt.np(alloc.dtype)
            out_names.append(name)
            out_avals.append(jax.core.ShapedArray(shape, dtype))
            zero_outs.append(np.zeros(shape, dtype))
    n_params = len(in_names)
    all_in_names = list(in_names) + list(out_names)
    if partition_name is not None:
        all_in_names.append(partition_name)

    def _body(*args):
        operands = list(args)
        if partition_name is not None:
            operands.append(bass2jax.partition_id_tensor())
        outs = bass2jax._bass_exec_p.bind(
            *operands,
            out_avals=tuple(out_avals),
            in_names=tuple(all_in_names),
            out_names=tuple(out_names),
            lowering_input_output_aliases=(),
            sim_require_finite=False,
            sim_require_nnan=False,
            nc=nc,
        )
        return tuple(outs)

    devices = jax.devices()[:NCORES]
    mesh = Mesh(np.asarray(devices), ("core",))
    n_outs = len(out_names)
    in_specs = (PartitionSpec("core"),) * (n_params + n_outs)
    out_specs = (PartitionSpec("core"),) * n_outs
    donate = tuple(range(n_params, n_params + n_outs))
    fn = jax.jit(
        shard_map(_body, mesh=mesh, in_specs=in_specs, out_specs=out_specs,
                  check_rep=False),
        donate_argnums=donate, keep_unused=True)
    _CACHE["runner"] = (fn, in_names, out_names, out_avals, zero_outs)
    return _CACHE["runner"]


def _concat_inputs(in_maps, in_names):
    return [np.concatenate([np.asarray(in_maps[c][nm]) for c in range(NCORES)],
                           axis=0) for nm in in_names]


def _run(in_maps):
    fn, in_names, out_names, out_avals, zero_outs = _get_runner()
    concat_in = _concat_inputs(in_maps, in_names)
    concat_zeros = [np.zeros((NCORES * z.shape[0], *z.shape[1:]), z.dtype)
                    for z in zero_outs]
    out_arrs = fn(*concat_in, *concat_zeros)
    return out_arrs, out_names, out_avals


def kernel(unaries, rgb):
    in_maps = _host_prepare(unaries, rgb)
    out_arrs, out_names, out_avals = _run(in_maps)
    qi = out_names.index("qout")
    q = np.asarray(out_arrs[qi]).reshape(NCORES, S, L).reshape(N, L)
    return np.ascontiguousarray(q[None].astype(np.float32))


def time_kernel(unaries, rgb, iters=20):
    """Steady-state per-call wall time of the compiled 8-core executable,
    with inputs pre-staged on device."""
    import time as _time
    import jax
    in_maps = _host_prepare(unaries, rgb)
    fn, in_names, out_names, out_avals, zero_outs = _get_runner()
    concat_in = _concat_inputs(in_maps, in_names)

    def once():
        concat_zeros = [np.zeros((NCORES * z.shape[0], *z.shape[1:]), z.dtype)
                        for z in zero_outs]
        outs = fn(*concat_in, *concat_zeros)
        jax.block_until_ready(outs)
        return outs

    once()  # warm
    times = []
    for _ in range(iters):
        t0 = _time.perf_counter()
        once()
        times.append(_time.perf_counter() - t0)
    return min(times), sorted(times)[len(times) // 2]


# revision 59
# speedup vs baseline: 1.9536x; 1.0708x over previous
"""Trainium2 Bass kernel: dense-CRF mean-field layer (96x96 image, 21 labels).

Strategy (8 NeuronCores, row-sharded, K-stationary form):
  * Bilateral kernel K_bl [N,N] is built once on-device (fused feature matmul
    + exp) in bf16 and stays SBUF-resident per core as its [all j, own i]
    slice.
  * The per-iteration bilateral message uses K_bl tiles as the STATIONARY
    matmul operand and streams q chunks [128, 22] as the moving operand:
    out[96 own-pixels, 22] accumulates over 72 j-chunks.  Output is
    pixel-major, so the softmax/combine chain runs directly on [x, y, l]
    tiles with no transposes.
  * Spatial kernel is separable: y-blur is done with t1 (x-blurred q from the
    previous iteration, all-gathered) as the stationary operand per label,
    x-blur per own row after the softmax.  W_SPATIAL/norm folded into the
    host-prepared blur matrices.
  * Per iteration the new q is written straight into the all-gather payload
    (6 partition-shift DMA pieces), together with the x-blurred t1.
"""
import sys
sys.path.insert(0, "/opt/trn_rl_repo")
import os
import numpy as np
import ml_dtypes

H = W = 96
N = H * W                  # 9216
L = 21
LE = L + 1                 # 22 channels (21 labels + norm channel)
ALPHA, BETA, GAMMA = 80.0, 13.0, 3.0
W_SPATIAL, W_BILATERAL = 3.0, 10.0
NUM_ITERATIONS = 5
NCORES = 8
S = N // NCORES            # 1152 rows per core
YPC = H // NCORES          # 12 image rows per core
CH = N // 128              # 72 chunks of 128 rows (global j)
KCOLS = CH * S             # 82944 K_bl sbuf columns (bf16)
QCOLS = CH * LE            # 1584
PAYQ_F32 = 128 * 9 * LE // 2   # 12672 f32 slots holding the bf16 q-part
PAYT = S * LE // 2             # 12672 f32 slots holding the bf16 t1 part
PAY = PAYQ_F32 + PAYT          # 25344
ONESV = 0.1                # q norm-channel value => reciprocal gives 10/norm

# Schraudolph bf16 exp: bits_u16 = trunc(A_SCH * max(x + SH_SCH, 0)),
# bitcast as bf16 ~= exp(x) (max rel err 3.3%, C=5 fitted numerically)
A_SCH = 128.0 / 0.6931471805599453
SH_SCH = (16256.0 - 5.0) / A_SCH

# partition-shift piece groups for the 96->128 repack of q into the payload:
# maps qyb[x0:x0+n, yi, yo, l] -> q128[p0:p0+n, ao, yi, l]  (ai == yi)
# constraint: yo*96 == ao*128 - x0 + p0
QPIECES = ((0, 96, 0, 0, 0),
           (0, 96, 32, 3, 2),
           (0, 32, 96, 1, 0),
           (32, 64, 0, 1, 1),
           (0, 64, 64, 2, 1),
           (64, 32, 0, 2, 2))

LAST_EXEC_NS = None
_CACHE = {}


def _build_bass(sim1=False):
    """Build the kernel. sim1=True builds a single-core variant where the
    AllGather is replaced by 8 local DRAM copies (for TimelineSim analysis)."""
    key = "nc_sim1" if sim1 else "nc"
    if key in _CACHE:
        return _CACHE[key]
    import concourse.bass as bass  # noqa: F401
    from concourse import bacc
    import concourse.mybir as mybir
    import concourse.tile as tile

    f32 = mybir.dt.float32
    bf16 = mybir.dt.bfloat16
    AF = mybir.ActivationFunctionType
    OP = mybir.AluOpType
    AX = mybir.AxisListType

    dbg = bool(int(os.environ.get("CRF_DEBUG", "0"))) and not sim1
    nc = bacc.Bacc("TRN2", target_bir_lowering=False, debug=False,
                   num_devices=1 if sim1 else NCORES)

    featL_d = nc.dram_tensor("featL", [21, N], bf16, kind="ExternalInput")
    featR_d = nc.dram_tensor("featR", [21, S], bf16, kind="ExternalInput")
    uSB_d = nc.dram_tensor("uSB", [W, YPC * L], f32, kind="ExternalInput")
    Ax_d = nc.dram_tensor("Ax", [W, W], bf16, kind="ExternalInput")
    Ay_d = nc.dram_tensor("Ay", [H, YPC], bf16, kind="ExternalInput")
    qsb0_d = nc.dram_tensor("qsb0", [128, QCOLS], bf16, kind="ExternalInput")
    t1f0_d = nc.dram_tensor("t1f0", [H, LE * W], bf16, kind="ExternalInput")
    qout_d = nc.dram_tensor("qout", [S, L], f32, kind="ExternalOutput")
    if dbg:
        dbg_kbl = nc.dram_tensor("dbg_kbl", [128, S], bf16, kind="ExternalOutput")
        dbg_pbl = nc.dram_tensor("dbg_pbl", [W, YPC * LE], f32, kind="ExternalOutput")
        dbg_v = nc.dram_tensor("dbg_v", [W, YPC * L], f32, kind="ExternalOutput")
        dbg_lg = nc.dram_tensor("dbg_lg", [W, YPC * L], f32, kind="ExternalOutput")
        dbg_qy = nc.dram_tensor("dbg_qy", [W, YPC * L], f32, kind="ExternalOutput")
        dbg_t1t = nc.dram_tensor("dbg_t1t", [LE, S], bf16, kind="ExternalOutput")

    # combine groups: rows with y % 4 == yo finish together so the payload
    # piece(s) for that yo can fire while later groups still accumulate
    YGROUPS = [(yo, [yo, yo + 4, yo + 8]) for yo in range(4)]

    with tile.TileContext(nc) as tc:
        with (
            tc.tile_pool(name="const", bufs=1) as constp,
            tc.tile_pool(name="kbl", bufs=1) as kblp,
            tc.tile_pool(name="work", bufs=1) as work,
            tc.tile_pool(name="dram", bufs=2, space="DRAM") as dram,
        ):
            Ax = constp.tile([W, W], bf16)
            Ay = constp.tile([H, YPC], bf16)
            uSB = constp.tile([W, YPC * L], f32)
            Kbl = kblp.tile([128, KCOLS], bf16)
            qsb = work.tile([128, QCOLS], bf16, tag="qsb", bufs=2)
            t1full = work.tile([H, LE * W], bf16, tag="t1full", bufs=2)

            # ---------- precompute K_bl = exp(-||g_i - g_j||^2 / 2) ----------
            # Elementwise exp split across ACT (table exp) and DVE/Pool
            # (Schraudolph bf16-bitcast exp) over 512-col PSUM windows; the
            # 8-deep window ring lets all three engines run concurrently.
            u16 = mybir.dt.uint16
            # A: ACT table-exp from PSUM.  P: DVE shift+clamp PSUM->SBUF f32,
            # then Pool scale+u16-convert SBUF->SBUF (GPSIMD can't touch
            # PSUM).  D: DVE does both steps.
            WPAT = "AADAADAADAADA"  # ACT + DVE Schraudolph split (per-1024)
            with (
                tc.tile_pool(name="pre_sb", bufs=2) as pre_sb,
                tc.tile_pool(name="stg", bufs=2) as stgp,
                tc.tile_pool(name="featRp", bufs=1) as featRp,
                tc.tile_pool(name="pre_ps", bufs=4, space="PSUM") as pre_ps,
            ):
                featR = featRp.tile([21, S], bf16)
                nc.sync.dma_start(featR[:], featR_d[:])
                flb, flb_idx = None, -1
                WIN = 1024
                NW = KCOLS // WIN
                for wdx in range(NW):
                    if wdx == 1:
                        # late-need loads, queued behind featR + first flb
                        nc.sync.dma_start(qsb[:], qsb0_d[:])
                        nc.sync.dma_start(t1full[:], t1f0_d[:])
                    if wdx == 2:
                        nc.sync.dma_start(Ax[:], Ax_d[:])
                        nc.sync.dma_start(Ay[:], Ay_d[:])
                        nc.sync.dma_start(uSB[:], uSB_d[:])
                    g0 = wdx * WIN
                    d2 = pre_ps.tile([128, WIN], f32, tag="d2")
                    cuts = sorted({g0, g0 + WIN}
                                  | set(range((g0 // 512 + 1) * 512,
                                              g0 + WIN, 512))
                                  | set(range((g0 // S + 1) * S,
                                              g0 + WIN, S)))
                    for a, b in zip(cuts[:-1], cuts[1:]):
                        ch = a // S
                        if ch // 8 != flb_idx:
                            flb_idx = ch // 8
                            flb = pre_sb.tile([21, 1024], bf16, tag="fl")
                            nc.sync.dma_start(
                                flb[:],
                                featL_d[:, flb_idx * 1024:(flb_idx + 1) * 1024])
                        nc.tensor.matmul(
                            d2[:, a - g0:b - g0],
                            flb[:, (ch % 8) * 128:(ch % 8 + 1) * 128],
                            featR[:, a - ch * S:b - ch * S],
                            start=True, stop=True)
                    e = WPAT[wdx % len(WPAT)]
                    if e == "A":
                        nc.scalar.activation(Kbl[:, g0:g0 + WIN],
                                             d2[:, 0:WIN], AF.Exp)
                    else:
                        nc.vector.tensor_scalar(d2[:, 0:WIN], d2[:, 0:WIN],
                                                SH_SCH, 0.0,
                                                op0=OP.add, op1=OP.max)
                        nc.vector.tensor_scalar(
                            Kbl[:, g0:g0 + WIN].bitcast(u16),
                            d2[:, 0:WIN], A_SCH, None, op0=OP.mult)

            if dbg:
                nc.sync.dma_start(dbg_kbl.ap(), Kbl[:, 0:S])

            # ---------- mean-field iterations ----------
            psBL_ctx = tc.tile_pool(name="psBL", bufs=1, space="PSUM")
            psBL = psBL_ctx.__enter__()
            psSP_ctx = tc.tile_pool(name="psSP", bufs=1, space="PSUM")
            psSP = psSP_ctx.__enter__()
            psXB_ctx = tc.tile_pool(name="psXB", bufs=2, space="PSUM")
            psXB = psXB_ctx.__enter__()
            psWM_ctx = tc.tile_pool(name="psWM", bufs=1, space="PSUM")
            psWM = psWM_ctx.__enter__()

            def pe_warm_fillers(n, dep_kbl=False):
                """Dummy 512-col matmuls that keep the tensor engine's
                p-state ramp alive across DMA-bound stretches.  With
                dep_kbl, filler k reads a late K_bl window so the stream
                paces itself to the end of the build."""
                wm = psWM.tile([LE, 512], f32, tag="warm")
                for k in range(n):
                    w = (NW - n + k) if dep_kbl else (k % 64)
                    nc.tensor.matmul(wm[:], qsb[:, 0:LE],
                                     Kbl[:, w * 512:(w + 1) * 512],
                                     start=True, stop=True)

            # pe_warm_fillers(12, dep_kbl=True)
            qag_prev = None
            for it in range(NUM_ITERATIONS):
                last = it == NUM_ITERATIONS - 1
                if it > 0:
                    qsb = work.tile([128, QCOLS], bf16, tag="qsb", bufs=2)
                    t1full = work.tile([H, LE * W], bf16, tag="t1full",
                                       bufs=2)
                    TB = NCORES * PAYQ_F32

                    def t1recv(r, eng):
                        tsrc = (qag_prev[0:1, TB + r * PAYT:
                                         TB + (r + 1) * PAYT].bitcast(bf16)
                                .rearrange("a b -> (a b)")
                                .rearrange("(l y x) -> y l x",
                                           l=LE, y=YPC, x=W))
                        tdst = (t1full[r * YPC:(r + 1) * YPC, :]
                                .rearrange("y (l x) -> y l x", l=LE, x=W))
                        eng.dma_start(tdst, tsrc)
                    t1recv(0, nc.sync)
                    for h in (0, 1):
                        qsrc = (qag_prev[0:1, h * 4 * PAYQ_F32:
                                         (h + 1) * 4 * PAYQ_F32]
                                .bitcast(bf16)
                                .rearrange("a (r p c) -> p (a r) c",
                                           r=4, p=128))
                        nc.sync.dma_start(
                            qsb[:].rearrange("p (r c) -> p r c", r=NCORES)
                            [:, h * 4:(h + 1) * 4], qsrc)
                    t1recv(1, nc.sync)
                    for r in range(2, NCORES):
                        t1recv(r, nc.scalar if r % 2 else nc.gpsimd)

                # spatial y-blur: per label, strided t1 slice stationary
                sp = psSP.tile([W, L * YPC], f32, tag="sp", bufs=2)
                for lb in range(L):
                    nc.tensor.matmul(sp[:, lb * YPC:(lb + 1) * YPC],
                                     t1full[:, lb * W:(lb + 1) * W], Ay[:],
                                     start=True, stop=True)
                v = work.tile([W, YPC * L], f32, tag="v", bufs=2)
                v3 = v[:].rearrange("x (y l) -> x y l", l=L)
                nc.vector.tensor_tensor(
                    v3, uSB[:].rearrange("x (y l) -> x y l", l=L),
                    sp[:].rearrange("x (l y) -> x y l", l=L), OP.add)
                pbl = psBL.tile([W, YPC * LE], f32, tag="pbl", bufs=2)

                lg = work.tile([W, YPC * L], f32, tag="lg", bufs=2)
                lg3 = lg[:].rearrange("x (y l) -> x y l", l=L)
                qy = work.tile([W, YPC * L], f32, tag="qy", bufs=2)
                qy3 = qy[:].rearrange("x (y l) -> x y l", l=L)
                ssum = work.tile([W, YPC], f32, tag="ssum", bufs=2)
                rec = work.tile([W, YPC], f32, tag="rec", bufs=2)
                if last:
                    qyf = work.tile([W, YPC * LE], f32, tag="qyf")
                    qyf3 = qyf[:].rearrange("x (y l) -> x y l", l=LE)
                else:
                    # per-group q tiles keep the payload-piece DMA deps
                    # narrow (whole-tile tracking would defer every piece
                    # to the last group's softmax)
                    qybs = [work.tile([W, 3 * LE], bf16, tag=f"qyb{g}",
                                      name=f"qyb{g}", bufs=2)
                            for g in range(4)]
                    t1X = work.tile([LE, S], bf16, tag="t1X", bufs=2)
                    pl = dram.tile([1, PAY], f32, tag="pl")
                    plq = (pl[0:1, 0:PAYQ_F32].bitcast(bf16)
                           .rearrange("a (p ai ao l) -> (a p) ao ai l",
                                      p=128, ai=3, ao=3))
                pbl3 = pbl[:].rearrange("x (y l) -> x y l", l=LE)

                def emit_xblur(yo):
                    # x-blur rows y%4==yo (new q stationary, Ax moving)
                    gsl = slice(yo, yo + 9, 4)
                    xb = psXB.tile([LE, 3 * W], f32, tag="xb")
                    for k, r in enumerate((yo, yo + 4, yo + 8)):
                        nc.tensor.matmul(xb[:, k * W:(k + 1) * W],
                                         qybs[yo][:, k * LE:(k + 1) * LE],
                                         Ax[:], start=True, stop=True)
                    nc.scalar.copy(
                        t1X[:].rearrange("l (y x) -> l y x", x=W)[:, gsl],
                        xb[:].rearrange("l (k x) -> l k x", x=W))

                for (yo, ys) in YGROUPS:
                    for r in ys:
                        for ch in range(CH):
                            nc.tensor.matmul(
                                pbl[:, r * LE:(r + 1) * LE],
                                Kbl[:, ch * S + r * W: ch * S + (r + 1) * W],
                                qsb[:, ch * LE:(ch + 1) * LE],
                                start=(ch == 0), stop=(ch == CH - 1))
                    # previous group's x-blur: emitted here so the PE never
                    # waits on the previous group's softmax chain
                    if yo > 0 and not last:
                        emit_xblur(yo - 1)
                    # combine + softmax for this group's 3 rows
                    gsl = slice(yo, yo + 9, 4)  # rows yo, yo+4, yo+8
                    nc.vector.reciprocal(rec[:, gsl][:, :, None],
                                         pbl3[:, gsl, L:LE])
                    nc.vector.tensor_tensor(
                        lg3[:, gsl], pbl3[:, gsl, 0:L],
                        rec[:, gsl][:, :, None].to_broadcast([W, 3, L]),
                        OP.mult)
                    nc.vector.tensor_tensor(lg3[:, gsl], lg3[:, gsl],
                                            v3[:, gsl], OP.add)
                    nc.scalar.activation(qy3[:, gsl], lg3[:, gsl], AF.Exp)
                    nc.vector.reduce_sum(ssum[:, gsl], qy3[:, gsl], axis=AX.X)
                    nc.vector.reciprocal(ssum[:, gsl], ssum[:, gsl])
                    qt = (qyf3[:, gsl] if last
                          else qybs[yo][:].rearrange("x (k l) -> x k l", l=LE))
                    nc.vector.tensor_tensor(
                        qt[:, :, 0:L], qy3[:, gsl],
                        ssum[:, gsl][:, :, None].to_broadcast([W, 3, L]),
                        OP.mult)
                    if last:
                        continue
                    if it < 2:
                        nc.vector.memset(qt[:, :, L:LE], ONESV)
                    # payload piece(s) for this yo straight into DRAM;
                    # the last group's piece goes on SP so it lands just
                    # before the q-section gather on the same queue
                    for (x0, n, p0, pyo, ao) in QPIECES:
                        if pyo == yo:
                            eng = nc.sync if yo == 3 else nc.gpsimd
                            eng.dma_start(
                                plq[p0:p0 + n, ao],
                                qybs[yo][x0:x0 + n, :]
                                .rearrange("x (k l) -> x k l", l=LE))
                if not last:
                    emit_xblur(3)

                if dbg and it == 0:
                    nc.sync.dma_start(dbg_pbl.ap(), pbl[:])
                    nc.sync.dma_start(dbg_lg.ap(), lg[:])
                if dbg and it == 1:
                    nc.sync.dma_start(dbg_qy.ap(), qy[:])
                    nc.sync.dma_start(dbg_t1t.ap(), t1X[:])

                if last:
                    nc.sync.dma_start(
                        qout_d.ap().rearrange("(y x) l -> x y l", x=W),
                        qyf3[:, :, 0:L])
                    continue

                # t1 payload part (layout (l, y, x) per core)
                nc.scalar.dma_start(
                    pl[0:1, PAYQ_F32:PAY].bitcast(bf16)
                      .rearrange("a (l c) -> (a l) c", l=LE),
                    t1X[:])
                # AllGather split by payload section into a contiguous
                # section-major gather buffer [q sections | t1 sections]:
                # the q gather only waits on the q pieces, so the qsb chain
                # starts while the t1 payload is still being written
                qag = dram.tile([1, NCORES * PAY], f32, tag="qag")
                if sim1:
                    nc.sync.dma_start(
                        qag[0:1, 0:NCORES * PAYQ_F32],
                        pl[0:1, 0:PAYQ_F32].to_broadcast([NCORES, PAYQ_F32]))
                    nc.scalar.dma_start(
                        qag[0:1, NCORES * PAYQ_F32:],
                        pl[0:1, PAYQ_F32:PAY].to_broadcast([NCORES, PAYT]))
                else:
                    nc.gpsimd.collective_compute(
                        "AllGather", OP.bypass,
                        replica_groups=[list(range(NCORES))],
                        ins=[pl[0:1, 0:PAYQ_F32].opt()],
                        outs=[qag[0:1, 0:NCORES * PAYQ_F32].opt()])
                    nc.gpsimd.collective_compute(
                        "AllGather", OP.bypass,
                        replica_groups=[list(range(NCORES))],
                        ins=[pl[0:1, PAYQ_F32:PAY].opt()],
                        outs=[qag[0:1, NCORES * PAYQ_F32:].opt()])
                qag_prev = qag
            psWM_ctx.__exit__(None, None, None)
            psXB_ctx.__exit__(None, None, None)
            psSP_ctx.__exit__(None, None, None)
            psBL_ctx.__exit__(None, None, None)

    nc.compile()
    _CACHE[key] = nc
    return nc


def _host_prepare(unaries, rgb):
    u = np.asarray(unaries, np.float32).reshape(N, L)
    c = np.asarray(rgb, np.float32).reshape(N, 3)

    ys, xs = np.meshgrid(np.arange(H, dtype=np.float64),
                         np.arange(W, dtype=np.float64), indexing="ij")
    pos = np.stack([ys.ravel(), xs.ravel()], -1)            # [N, 2]
    g = np.concatenate([c.astype(np.float64) / BETA, pos / ALPHA], 1)
    g = g - g.mean(0, keepdims=True)
    sq = (g * g).sum(1)
    ones = np.ones(N, np.float64)
    L7 = np.concatenate([g.T, ones[None], (-0.5 * sq)[None]], 0)  # [7, N] j
    R7 = np.concatenate([g.T, (-0.5 * sq)[None], ones[None]], 0)  # [7, N] i
    bfd = ml_dtypes.bfloat16
    Lhi = L7.astype(bfd)
    Llo = (L7 - Lhi.astype(np.float64)).astype(bfd)
    Rhi = R7.astype(bfd)
    Rlo = (R7 - Rhi.astype(np.float64)).astype(bfd)
    # dot = Lhi.Rhi + Lhi.Rlo + Llo.Rhi  (Llo.Rlo dropped, ~1e-3)
    featL = np.ascontiguousarray(np.concatenate([Lhi, Lhi, Llo], 0))  # [21,N]
    featR = np.ascontiguousarray(np.concatenate([Rhi, Rlo, Rhi], 0))  # [21,N]

    d = np.arange(W, dtype=np.float64)
    A = np.exp(-(d[:, None] - d[None, :]) ** 2 / (2.0 * GAMMA * GAMMA))
    nvec = A.sum(0)
    Ax = np.ascontiguousarray((A / nvec[None, :]).astype(ml_dtypes.bfloat16))

    um = u.max(1, keepdims=True)
    e = np.exp(u - um)
    q0 = e / e.sum(1, keepdims=True)
    q0e = np.concatenate([q0, np.full((N, 1), ONESV, np.float32)], 1)  # [N,22]
    qsb0 = np.ascontiguousarray(
        q0e.reshape(CH, 128, LE).transpose(1, 0, 2).reshape(128, QCOLS)
    ).astype(ml_dtypes.bfloat16)

    q3 = q0e.reshape(H, W, LE).astype(np.float64)
    t1 = np.einsum("Xx,yXl->ylx", A / nvec[None, :], q3)      # [96, 22, 96]
    t1f0 = np.ascontiguousarray(t1.reshape(H, LE * W).astype(ml_dtypes.bfloat16))

    in_maps = []
    for core in range(NCORES):
        rows = slice(core * S, (core + 1) * S)
        uSB_c = np.ascontiguousarray(
            u[rows].reshape(YPC, W, L).transpose(1, 0, 2).reshape(W, YPC * L))
        yc = slice(core * YPC, (core + 1) * YPC)
        Ay_c = np.ascontiguousarray(
            (A[:, yc] * (W_SPATIAL / nvec[yc])[None, :]).astype(ml_dtypes.bfloat16))
        in_maps.append({
            "featL": featL,
            "featR": np.ascontiguousarray(featR[:, rows]),
            "uSB": uSB_c,
            "Ax": Ax,
            "Ay": Ay_c,
            "qsb0": qsb0,
            "t1f0": t1f0,
        })
    return in_maps


def _get_runner():
    """Compile once; return (fn, in_names, out_names) where fn maps
    concatenated global numpy inputs -> list of per-core output dicts."""
    if "runner" in _CACHE:
        return _CACHE["runner"]
    import jax
    from jax.sharding import Mesh, PartitionSpec
    from jax.experimental.shard_map import shard_map
    import concourse.mybir as mybir
    from concourse import bass2jax

    nc = _build_bass()
    bass2jax.install_neuronx_cc_hook()

    partition_name = (nc.partition_id_tensor.name
                      if nc.partition_id_tensor else None)
    in_names, out_names, out_avals, zero_outs = [], [], [], []
    for alloc in nc.m.functions[0].allocations:
        if not isinstance(alloc, mybir.MemoryLocationSet):
            continue
        name = alloc.memorylocations[0].name
        if alloc.kind == "ExternalInput":
            if name != partition_name:
                in_names.append(name)
        elif alloc.kind == "ExternalOutput":
            shape = tuple(alloc.tensor_shape)
            dtype = mybir.dt.np(alloc.dtype)
            out_names.append(name)
            out_avals.append(jax.core.ShapedArray(shape, dtype))
            zero_outs.append(np.zeros(shape, dtype))
    n_params = len(in_names)
    all_in_names = list(in_names) + list(out_names)
    if partition_name is not None:
        all_in_names.append(partition_name)

    def _body(*args):
        operands = list(args)
        if partition_name is not None:
            operands.append(bass2jax.partition_id_tensor())
        outs = bass2jax._bass_exec_p.bind(
            *operands,
            out_avals=tuple(out_avals),
            in_names=tuple(all_in_names),
            out_names=tuple(out_names),
            lowering_input_output_aliases=(),
            sim_require_finite=False,
            sim_require_nnan=False,
            nc=nc,
        )
        return tuple(outs)

    devices = jax.devices()[:NCORES]
    mesh = Mesh(np.asarray(devices), ("core",))
    n_outs = len(out_names)
    in_specs = (PartitionSpec("core"),) * (n_params + n_outs)
    out_specs = (PartitionSpec("core"),) * n_outs
    donate = tuple(range(n_params, n_params + n_outs))
    fn = jax.jit(
        shard_map(_body, mesh=mesh, in_specs=in_specs, out_specs=out_specs,
                  check_rep=False),
        donate_argnums=donate, keep_unused=True)
    _CACHE["runner"] = (fn, in_names, out_names, out_avals, zero_outs)
    return _CACHE["runner"]


def _concat_inputs(in_maps, in_names):
    return [np.concatenate([np.asarray(in_maps[c][nm]) for c in range(NCORES)],
                           axis=0) for nm in in_names]


def _run(in_maps):
    fn, in_names, out_names, out_avals, zero_outs = _get_runner()
    concat_in = _concat_inputs(in_maps, in_names)
    concat_zeros = [np.zeros((NCORES * z.shape[0], *z.shape[1:]), z.dtype)
                    for z in zero_outs]
    out_arrs = fn(*concat_in, *concat_zeros)
    return out_arrs, out_names, out_avals


def kernel(unaries, rgb):
    in_maps = _host_prepare(unaries, rgb)
    out_arrs, out_names, out_avals = _run(in_maps)
    qi = out_names.index("qout")
    q = np.asarray(out_arrs[qi]).reshape(NCORES, S, L).reshape(N, L)
    return np.ascontiguousarray(q[None].astype(np.float32))


def time_kernel(unaries, rgb, iters=20):
    """Steady-state per-call wall time of the compiled 8-core executable,
    with inputs pre-staged on device."""
    import time as _time
    import jax
    in_maps = _host_prepare(unaries, rgb)
    fn, in_names, out_names, out_avals, zero_outs = _get_runner()
    concat_in = _concat_inputs(in_maps, in_names)

    def once():
        concat_zeros = [np.zeros((NCORES * z.shape[0], *z.shape[1:]), z.dtype)
                        for z in zero_outs]
        outs = fn(*concat_in, *concat_zeros)
        jax.block_until_ready(outs)
        return outs

    once()  # warm
    times = []
    for _ in range(iters):
        t0 = _time.perf_counter()
        once()
        times.append(_time.perf_counter() - t0)
    return min(times), sorted(times)[len(times) // 2]


# revision 62
# speedup vs baseline: 1.9886x; 1.0179x over previous
"""Trainium2 Bass kernel: dense-CRF mean-field layer (96x96 image, 21 labels).

Strategy (8 NeuronCores, row-sharded, K-stationary form):
  * Bilateral kernel K_bl [N,N] is built once on-device (fused feature matmul
    + exp) in bf16 and stays SBUF-resident per core as its [all j, own i]
    slice.
  * The per-iteration bilateral message uses K_bl tiles as the STATIONARY
    matmul operand and streams q chunks [128, 22] as the moving operand:
    out[96 own-pixels, 22] accumulates over 72 j-chunks.  Output is
    pixel-major, so the softmax/combine chain runs directly on [x, y, l]
    tiles with no transposes.
  * Spatial kernel is separable: y-blur is done with t1 (x-blurred q from the
    previous iteration, all-gathered) as the stationary operand per label,
    x-blur per own row after the softmax.  W_SPATIAL/norm folded into the
    host-prepared blur matrices.
  * Per iteration the new q is written straight into the all-gather payload
    (6 partition-shift DMA pieces), together with the x-blurred t1.
"""
import sys
sys.path.insert(0, "/opt/trn_rl_repo")
import os
import numpy as np
import ml_dtypes

H = W = 96
N = H * W                  # 9216
L = 21
LE = L + 1                 # 22 channels (21 labels + norm channel)
ALPHA, BETA, GAMMA = 80.0, 13.0, 3.0
W_SPATIAL, W_BILATERAL = 3.0, 10.0
NUM_ITERATIONS = 5
NCORES = 8
S = N // NCORES            # 1152 rows per core
YPC = H // NCORES          # 12 image rows per core
CH = N // 128              # 72 chunks of 128 rows (global j)
KCOLS = CH * S             # 82944 K_bl sbuf columns (bf16)
QCOLS = CH * LE            # 1584
PAYQ_F32 = 128 * 9 * LE // 2   # 12672 f32 slots holding the bf16 q-part
PAYT = S * LE // 2             # 12672 f32 slots holding the bf16 t1 part
PAY = PAYQ_F32 + PAYT          # 25344
ONESV = 0.1                # q norm-channel value => reciprocal gives 10/norm

# Schraudolph bf16 exp: bits_u16 = trunc(A_SCH * max(x + SH_SCH, 0)),
# bitcast as bf16 ~= exp(x) (max rel err 3.3%, C=5 fitted numerically)
A_SCH = 128.0 / 0.6931471805599453
SH_SCH = (16256.0 - 5.0) / A_SCH

# partition-shift piece groups for the 96->128 repack of q into the payload:
# maps qyb[x0:x0+n, yi, yo, l] -> q128[p0:p0+n, ao, yi, l]  (ai == yi)
# constraint: yo*96 == ao*128 - x0 + p0
QPIECES = ((0, 96, 0, 0, 0),
           (0, 96, 32, 3, 2),
           (0, 32, 96, 1, 0),
           (32, 64, 0, 1, 1),
           (0, 64, 64, 2, 1),
           (64, 32, 0, 2, 2))

LAST_EXEC_NS = None
_CACHE = {}


def _build_bass(sim1=False):
    """Build the kernel. sim1=True builds a single-core variant where the
    AllGather is replaced by 8 local DRAM copies (for TimelineSim analysis)."""
    key = "nc_sim1" if sim1 else "nc"
    if key in _CACHE:
        return _CACHE[key]
    import concourse.bass as bass  # noqa: F401
    from concourse import bacc
    import concourse.mybir as mybir
    import concourse.tile as tile

    f32 = mybir.dt.float32
    bf16 = mybir.dt.bfloat16
    AF = mybir.ActivationFunctionType
    OP = mybir.AluOpType
    AX = mybir.AxisListType

    dbg = bool(int(os.environ.get("CRF_DEBUG", "0"))) and not sim1
    nc = bacc.Bacc("TRN2", target_bir_lowering=False, debug=False,
                   num_devices=1 if sim1 else NCORES)

    featL_d = nc.dram_tensor("featL", [21, N], bf16, kind="ExternalInput")
    featR_d = nc.dram_tensor("featR", [21, S], bf16, kind="ExternalInput")
    uSB_d = nc.dram_tensor("uSB", [W, YPC * L], f32, kind="ExternalInput")
    Ax_d = nc.dram_tensor("Ax", [W, W], bf16, kind="ExternalInput")
    Ay_d = nc.dram_tensor("Ay", [H, YPC], bf16, kind="ExternalInput")
    qsb0_d = nc.dram_tensor("qsb0", [128, QCOLS], bf16, kind="ExternalInput")
    t1f0_d = nc.dram_tensor("t1f0", [H, LE * W], bf16, kind="ExternalInput")
    qout_d = nc.dram_tensor("qout", [S, L], f32, kind="ExternalOutput")
    if dbg:
        dbg_kbl = nc.dram_tensor("dbg_kbl", [128, S], bf16, kind="ExternalOutput")
        dbg_pbl = nc.dram_tensor("dbg_pbl", [W, YPC * LE], f32, kind="ExternalOutput")
        dbg_v = nc.dram_tensor("dbg_v", [W, YPC * L], f32, kind="ExternalOutput")
        dbg_lg = nc.dram_tensor("dbg_lg", [W, YPC * L], f32, kind="ExternalOutput")
        dbg_qy = nc.dram_tensor("dbg_qy", [W, YPC * L], f32, kind="ExternalOutput")
        dbg_t1t = nc.dram_tensor("dbg_t1t", [LE, S], bf16, kind="ExternalOutput")

    # combine groups: rows with y % 4 == yo finish together so the payload
    # piece(s) for that yo can fire while later groups still accumulate
    YGROUPS = [(yo, [yo, yo + 4, yo + 8]) for yo in range(4)]

    with tile.TileContext(nc) as tc:
        with (
            tc.tile_pool(name="const", bufs=1) as constp,
            tc.tile_pool(name="kbl", bufs=1) as kblp,
            tc.tile_pool(name="work", bufs=1) as work,
            tc.tile_pool(name="dram", bufs=2, space="DRAM") as dram,
        ):
            Ax = constp.tile([W, W], bf16)
            Ay = constp.tile([H, YPC], bf16)
            uSB = constp.tile([W, YPC * L], f32)
            Kbl = kblp.tile([128, KCOLS], bf16)
            qsb = work.tile([128, QCOLS], bf16, tag="qsb", bufs=2)
            t1full = work.tile([H, LE * W], bf16, tag="t1full", bufs=2)

            # ---------- precompute K_bl = exp(-||g_i - g_j||^2 / 2) ----------
            # Elementwise exp split across ACT (table exp) and DVE/Pool
            # (Schraudolph bf16-bitcast exp) over 512-col PSUM windows; the
            # 8-deep window ring lets all three engines run concurrently.
            u16 = mybir.dt.uint16
            # A: ACT table-exp from PSUM.  P: DVE shift+clamp PSUM->SBUF f32,
            # then Pool scale+u16-convert SBUF->SBUF (GPSIMD can't touch
            # PSUM).  D: DVE does both steps.
            WPAT = "APAPAPAPAPAPAPAPAPAPDAAAAAA"  # ACT/Pool/DVE 3-way split
            with (
                tc.tile_pool(name="pre_sb", bufs=2) as pre_sb,
                tc.tile_pool(name="stg", bufs=2) as stgp,
                tc.tile_pool(name="featRp", bufs=1) as featRp,
                tc.tile_pool(name="pre_ps", bufs=4, space="PSUM") as pre_ps,
            ):
                featR = featRp.tile([21, S], bf16)
                nc.sync.dma_start(featR[:], featR_d[:])
                flb, flb_idx = None, -1
                WIN = 1024
                NW = KCOLS // WIN
                for wdx in range(NW):
                    if wdx == 1:
                        # late-need loads, queued behind featR + first flb
                        nc.sync.dma_start(qsb[:], qsb0_d[:])
                        nc.sync.dma_start(t1full[:], t1f0_d[:])
                    if wdx == 2:
                        nc.sync.dma_start(Ax[:], Ax_d[:])
                        nc.sync.dma_start(Ay[:], Ay_d[:])
                        nc.sync.dma_start(uSB[:], uSB_d[:])
                    g0 = wdx * WIN
                    d2 = pre_ps.tile([128, WIN], f32, tag="d2")
                    cuts = sorted({g0, g0 + WIN}
                                  | set(range((g0 // 512 + 1) * 512,
                                              g0 + WIN, 512))
                                  | set(range((g0 // S + 1) * S,
                                              g0 + WIN, S)))
                    for a, b in zip(cuts[:-1], cuts[1:]):
                        ch = a // S
                        if ch // 8 != flb_idx:
                            flb_idx = ch // 8
                            flb = pre_sb.tile([21, 1024], bf16, tag="fl")
                            nc.sync.dma_start(
                                flb[:],
                                featL_d[:, flb_idx * 1024:(flb_idx + 1) * 1024])
                        nc.tensor.matmul(
                            d2[:, a - g0:b - g0],
                            flb[:, (ch % 8) * 128:(ch % 8 + 1) * 128],
                            featR[:, a - ch * S:b - ch * S],
                            start=True, stop=True)
                    e = WPAT[wdx % len(WPAT)]
                    if e == "A":
                        nc.scalar.activation(Kbl[:, g0:g0 + WIN],
                                             d2[:, 0:WIN], AF.Exp)
                    elif e == "D":
                        nc.vector.tensor_scalar(d2[:, 0:WIN], d2[:, 0:WIN],
                                                SH_SCH, 0.0,
                                                op0=OP.add, op1=OP.max)
                        nc.vector.tensor_scalar(
                            Kbl[:, g0:g0 + WIN].bitcast(u16),
                            d2[:, 0:WIN], A_SCH, None, op0=OP.mult)
                    else:
                        # DVE shifts/clamps PSUM->SBUF, Pool scales and
                        # converts SBUF->SBUF (GPSIMD cannot access PSUM)
                        stg = stgp.tile([128, WIN], f32, tag="stg")
                        nc.vector.tensor_scalar(stg[:], d2[:, 0:WIN],
                                                SH_SCH, 0.0,
                                                op0=OP.add, op1=OP.max)
                        nc.gpsimd.tensor_scalar(
                            Kbl[:, g0:g0 + WIN].bitcast(u16),
                            stg[:], A_SCH, None, op0=OP.mult)

            if dbg:
                nc.sync.dma_start(dbg_kbl.ap(), Kbl[:, 0:S])

            # ---------- mean-field iterations ----------
            psBL_ctx = tc.tile_pool(name="psBL", bufs=1, space="PSUM")
            psBL = psBL_ctx.__enter__()
            psSP_ctx = tc.tile_pool(name="psSP", bufs=1, space="PSUM")
            psSP = psSP_ctx.__enter__()
            psXB_ctx = tc.tile_pool(name="psXB", bufs=2, space="PSUM")
            psXB = psXB_ctx.__enter__()
            psWM_ctx = tc.tile_pool(name="psWM", bufs=1, space="PSUM")
            psWM = psWM_ctx.__enter__()

            def pe_warm_fillers(n, dep_kbl=False):
                """Dummy 512-col matmuls that keep the tensor engine's
                p-state ramp alive across DMA-bound stretches.  With
                dep_kbl, filler k reads a late K_bl window so the stream
                paces itself to the end of the build."""
                wm = psWM.tile([LE, 512], f32, tag="warm")
                for k in range(n):
                    w = (NW - n + k) if dep_kbl else (k % 64)
                    nc.tensor.matmul(wm[:], qsb[:, 0:LE],
                                     Kbl[:, w * 512:(w + 1) * 512],
                                     start=True, stop=True)

            # pe_warm_fillers(12, dep_kbl=True)
            qag_prev = None
            for it in range(NUM_ITERATIONS):
                last = it == NUM_ITERATIONS - 1
                if it > 0:
                    qsb = work.tile([128, QCOLS], bf16, tag="qsb", bufs=2)
                    t1full = work.tile([H, LE * W], bf16, tag="t1full",
                                       bufs=2)
                    TB = NCORES * PAYQ_F32

                    def t1recv(r, eng):
                        tsrc = (qag_prev[0:1, TB + r * PAYT:
                                         TB + (r + 1) * PAYT].bitcast(bf16)
                                .rearrange("a b -> (a b)")
                                .rearrange("(l y x) -> y l x",
                                           l=LE, y=YPC, x=W))
                        tdst = (t1full[r * YPC:(r + 1) * YPC, :]
                                .rearrange("y (l x) -> y l x", l=LE, x=W))
                        eng.dma_start(tdst, tsrc)
                    t1recv(0, nc.sync)
                    for h in (0, 1):
                        qsrc = (qag_prev[0:1, h * 4 * PAYQ_F32:
                                         (h + 1) * 4 * PAYQ_F32]
                                .bitcast(bf16)
                                .rearrange("a (r p c) -> p (a r) c",
                                           r=4, p=128))
                        nc.sync.dma_start(
                            qsb[:].rearrange("p (r c) -> p r c", r=NCORES)
                            [:, h * 4:(h + 1) * 4], qsrc)
                    t1recv(1, nc.sync)
                    for r in range(2, NCORES):
                        t1recv(r, nc.scalar if r % 2 else nc.gpsimd)

                # spatial y-blur: per label, strided t1 slice stationary
                sp = psSP.tile([W, L * YPC], f32, tag="sp", bufs=2)
                for lb in range(L):
                    nc.tensor.matmul(sp[:, lb * YPC:(lb + 1) * YPC],
                                     t1full[:, lb * W:(lb + 1) * W], Ay[:],
                                     start=True, stop=True)
                v = work.tile([W, YPC * L], f32, tag="v", bufs=2)
                v3 = v[:].rearrange("x (y l) -> x y l", l=L)
                nc.vector.tensor_tensor(
                    v3, uSB[:].rearrange("x (y l) -> x y l", l=L),
                    sp[:].rearrange("x (l y) -> x y l", l=L), OP.add)
                pbl = psBL.tile([W, YPC * LE], f32, tag="pbl", bufs=2)

                lg = work.tile([W, YPC * L], f32, tag="lg", bufs=2)
                lg3 = lg[:].rearrange("x (y l) -> x y l", l=L)
                qy = work.tile([W, YPC * L], f32, tag="qy", bufs=2)
                qy3 = qy[:].rearrange("x (y l) -> x y l", l=L)
                ssum = work.tile([W, YPC], f32, tag="ssum", bufs=2)
                rec = work.tile([W, YPC], f32, tag="rec", bufs=2)
                if last:
                    qyf = work.tile([W, YPC * LE], f32, tag="qyf")
                    qyf3 = qyf[:].rearrange("x (y l) -> x y l", l=LE)
                else:
                    # per-group q tiles keep the payload-piece DMA deps
                    # narrow (whole-tile tracking would defer every piece
                    # to the last group's softmax)
                    qybs = [work.tile([W, 3 * LE], bf16, tag=f"qyb{g}",
                                      name=f"qyb{g}", bufs=2)
                            for g in range(4)]
                    t1X = work.tile([LE, S], bf16, tag="t1X", bufs=2)
                    pl = dram.tile([1, PAY], f32, tag="pl")
                    plq = (pl[0:1, 0:PAYQ_F32].bitcast(bf16)
                           .rearrange("a (p ai ao l) -> (a p) ao ai l",
                                      p=128, ai=3, ao=3))
                pbl3 = pbl[:].rearrange("x (y l) -> x y l", l=LE)

                def emit_xblur(yo):
                    # x-blur rows y%4==yo (new q stationary, Ax moving)
                    gsl = slice(yo, yo + 9, 4)
                    xb = psXB.tile([LE, 3 * W], f32, tag="xb")
                    for k, r in enumerate((yo, yo + 4, yo + 8)):
                        nc.tensor.matmul(xb[:, k * W:(k + 1) * W],
                                         qybs[yo][:, k * LE:(k + 1) * LE],
                                         Ax[:], start=True, stop=True)
                    nc.scalar.copy(
                        t1X[:].rearrange("l (y x) -> l y x", x=W)[:, gsl],
                        xb[:].rearrange("l (k x) -> l k x", x=W))

                for (yo, ys) in YGROUPS:
                    for r in ys:
                        for ch in range(CH):
                            nc.tensor.matmul(
                                pbl[:, r * LE:(r + 1) * LE],
                                Kbl[:, ch * S + r * W: ch * S + (r + 1) * W],
                                qsb[:, ch * LE:(ch + 1) * LE],
                                start=(ch == 0), stop=(ch == CH - 1))
                    # previous group's x-blur: emitted here so the PE never
                    # waits on the previous group's softmax chain
                    if yo > 0 and not last:
                        emit_xblur(yo - 1)
                    # combine + softmax for this group's 3 rows
                    gsl = slice(yo, yo + 9, 4)  # rows yo, yo+4, yo+8
                    nc.vector.reciprocal(rec[:, gsl][:, :, None],
                                         pbl3[:, gsl, L:LE])
                    nc.vector.tensor_tensor(
                        lg3[:, gsl], pbl3[:, gsl, 0:L],
                        rec[:, gsl][:, :, None].to_broadcast([W, 3, L]),
                        OP.mult)
                    nc.vector.tensor_tensor(lg3[:, gsl], lg3[:, gsl],
                                            v3[:, gsl], OP.add)
                    nc.scalar.activation(qy3[:, gsl], lg3[:, gsl], AF.Exp)
                    nc.vector.reduce_sum(ssum[:, gsl], qy3[:, gsl], axis=AX.X)
                    nc.vector.reciprocal(ssum[:, gsl], ssum[:, gsl])
                    qt = (qyf3[:, gsl] if last
                          else qybs[yo][:].rearrange("x (k l) -> x k l", l=LE))
                    nc.vector.tensor_tensor(
                        qt[:, :, 0:L], qy3[:, gsl],
                        ssum[:, gsl][:, :, None].to_broadcast([W, 3, L]),
                        OP.mult)
                    if last:
                        continue
                    if it < 2:
                        nc.vector.memset(qt[:, :, L:LE], ONESV)
                    # payload piece(s) for this yo straight into DRAM;
                    # the last group's piece goes on SP so it lands just
                    # before the q-section gather on the same queue
                    for (x0, n, p0, pyo, ao) in QPIECES:
                        if pyo == yo:
                            eng = nc.sync if yo == 3 else nc.gpsimd
                            eng.dma_start(
                                plq[p0:p0 + n, ao],
                                qybs[yo][x0:x0 + n, :]
                                .rearrange("x (k l) -> x k l", l=LE))
                if not last:
                    emit_xblur(3)

                if dbg and it == 0:
                    nc.sync.dma_start(dbg_pbl.ap(), pbl[:])
                    nc.sync.dma_start(dbg_lg.ap(), lg[:])
                if dbg and it == 1:
                    nc.sync.dma_start(dbg_qy.ap(), qy[:])
                    nc.sync.dma_start(dbg_t1t.ap(), t1X[:])

                if last:
                    nc.sync.dma_start(
                        qout_d.ap().rearrange("(y x) l -> x y l", x=W),
                        qyf3[:, :, 0:L])
                    continue

                # t1 payload part (layout (l, y, x) per core)
                nc.scalar.dma_start(
                    pl[0:1, PAYQ_F32:PAY].bitcast(bf16)
                      .rearrange("a (l c) -> (a l) c", l=LE),
                    t1X[:])
                # AllGather split by payload section into a contiguous
                # section-major gather buffer [q sections | t1 sections]:
                # the q gather only waits on the q pieces, so the qsb chain
                # starts while the t1 payload is still being written
                qag = dram.tile([1, NCORES * PAY], f32, tag="qag")
                if sim1:
                    nc.sync.dma_start(
                        qag[0:1, 0:NCORES * PAYQ_F32],
                        pl[0:1, 0:PAYQ_F32].to_broadcast([NCORES, PAYQ_F32]))
                    nc.scalar.dma_start(
                        qag[0:1, NCORES * PAYQ_F32:],
                        pl[0:1, PAYQ_F32:PAY].to_broadcast([NCORES, PAYT]))
                else:
                    nc.gpsimd.collective_compute(
                        "AllGather", OP.bypass,
                        replica_groups=[list(range(NCORES))],
                        ins=[pl[0:1, 0:PAYQ_F32].opt()],
                        outs=[qag[0:1, 0:NCORES * PAYQ_F32].opt()])
                    nc.gpsimd.collective_compute(
                        "AllGather", OP.bypass,
                        replica_groups=[list(range(NCORES))],
                        ins=[pl[0:1, PAYQ_F32:PAY].opt()],
                        outs=[qag[0:1, NCORES * PAYQ_F32:].opt()])
                qag_prev = qag
            psWM_ctx.__exit__(None, None, None)
            psXB_ctx.__exit__(None, None, None)
            psSP_ctx.__exit__(None, None, None)
            psBL_ctx.__exit__(None, None, None)

    nc.compile()
    _CACHE[key] = nc
    return nc


def _host_prepare(unaries, rgb):
    u = np.asarray(unaries, np.float32).reshape(N, L)
    c = np.asarray(rgb, np.float32).reshape(N, 3)

    ys, xs = np.meshgrid(np.arange(H, dtype=np.float64),
                         np.arange(W, dtype=np.float64), indexing="ij")
    pos = np.stack([ys.ravel(), xs.ravel()], -1)            # [N, 2]
    g = np.concatenate([c.astype(np.float64) / BETA, pos / ALPHA], 1)
    g = g - g.mean(0, keepdims=True)
    sq = (g * g).sum(1)
    ones = np.ones(N, np.float64)
    L7 = np.concatenate([g.T, ones[None], (-0.5 * sq)[None]], 0)  # [7, N] j
    R7 = np.concatenate([g.T, (-0.5 * sq)[None], ones[None]], 0)  # [7, N] i
    bfd = ml_dtypes.bfloat16
    Lhi = L7.astype(bfd)
    Llo = (L7 - Lhi.astype(np.float64)).astype(bfd)
    Rhi = R7.astype(bfd)
    Rlo = (R7 - Rhi.astype(np.float64)).astype(bfd)
    # dot = Lhi.Rhi + Lhi.Rlo + Llo.Rhi  (Llo.Rlo dropped, ~1e-3)
    featL = np.ascontiguousarray(np.concatenate([Lhi, Lhi, Llo], 0))  # [21,N]
    featR = np.ascontiguousarray(np.concatenate([Rhi, Rlo, Rhi], 0))  # [21,N]

    d = np.arange(W, dtype=np.float64)
    A = np.exp(-(d[:, None] - d[None, :]) ** 2 / (2.0 * GAMMA * GAMMA))
    nvec = A.sum(0)
    Ax = np.ascontiguousarray((A / nvec[None, :]).astype(ml_dtypes.bfloat16))

    um = u.max(1, keepdims=True)
    e = np.exp(u - um)
    q0 = e / e.sum(1, keepdims=True)
    q0e = np.concatenate([q0, np.full((N, 1), ONESV, np.float32)], 1)  # [N,22]
    qsb0 = np.ascontiguousarray(
        q0e.reshape(CH, 128, LE).transpose(1, 0, 2).reshape(128, QCOLS)
    ).astype(ml_dtypes.bfloat16)

    q3 = q0e.reshape(H, W, LE).astype(np.float64)
    t1 = np.einsum("Xx,yXl->ylx", A / nvec[None, :], q3)      # [96, 22, 96]
    t1f0 = np.ascontiguousarray(t1.reshape(H, LE * W).astype(ml_dtypes.bfloat16))

    in_maps = []
    for core in range(NCORES):
        rows = slice(core * S, (core + 1) * S)
        uSB_c = np.ascontiguousarray(
            u[rows].reshape(YPC, W, L).transpose(1, 0, 2).reshape(W, YPC * L))
        yc = slice(core * YPC, (core + 1) * YPC)
        Ay_c = np.ascontiguousarray(
            (A[:, yc] * (W_SPATIAL / nvec[yc])[None, :]).astype(ml_dtypes.bfloat16))
        in_maps.append({
            "featL": featL,
            "featR": np.ascontiguousarray(featR[:, rows]),
            "uSB": uSB_c,
            "Ax": Ax,
            "Ay": Ay_c,
            "qsb0": qsb0,
            "t1f0": t1f0,
        })
    return in_maps


def _get_runner():
    """Compile once; return (fn, in_names, out_names) where fn maps
    concatenated global numpy inputs -> list of per-core output dicts."""
    if "runner" in _CACHE:
        return _CACHE["runner"]
    import jax
    from jax.sharding import Mesh, PartitionSpec
    from jax.experimental.shard_map import shard_map
    import concourse.mybir as mybir
    from concourse import bass2jax

    nc = _build_bass()
    bass2jax.install_neuronx_cc_hook()

    partition_name = (nc.partition_id_tensor.name
                      if nc.partition_id_tensor else None)
    in_names, out_names, out_avals, zero_outs = [], [], [], []
    for alloc in nc.m.functions[0].allocations:
        if not isinstance(alloc, mybir.MemoryLocationSet):
            continue
        name = alloc.memorylocations[0].name
        if alloc.kind == "ExternalInput":
            if name != partition_name:
                in_names.append(name)
        elif alloc.kind == "ExternalOutput":
            shape = tuple(alloc.tensor_shape)
            dtype = mybir.dt.np(alloc.dtype)
            out_names.append(name)
            out_avals.append(jax.core.ShapedArray(shape, dtype))
            zero_outs.append(np.zeros(shape, dtype))
    n_params = len(in_names)
    all_in_names = list(in_names) + list(out_names)
    if partition_name is not None:
        all_in_names.append(partition_name)

    def _body(*args):
        operands = list(args)
        if partition_name is not None:
            operands.append(bass2jax.partition_id_tensor())
        outs = bass2jax._bass_exec_p.bind(
            *operands,
            out_avals=tuple(out_avals),
            in_names=tuple(all_in_names),
            out_names=tuple(out_names),
            lowering_input_output_aliases=(),
            sim_require_finite=False,
            sim_require_nnan=False,
            nc=nc,
        )
        return tuple(outs)

    devices = jax.devices()[:NCORES]
    mesh = Mesh(np.asarray(devices), ("core",))
    n_outs = len(out_names)
    in_specs = (PartitionSpec("core"),) * (n_params + n_outs)
    out_specs = (PartitionSpec("core"),) * n_outs
    donate = tuple(range(n_params, n_params + n_outs))
    fn = jax.jit(
        shard_map(_body, mesh=mesh, in_specs=in_specs, out_specs=out_specs,
                  check_rep=False),
        donate_argnums=donate, keep_unused=True)
    _CACHE["runner"] = (fn, in_names, out_names, out_avals, zero_outs)
    return _CACHE["runner"]


def _concat_inputs(in_maps, in_names):
    return [np.concatenate([np.asarray(in_maps[c][nm]) for c in range(NCORES)],
                           axis=0) for nm in in_names]


def _run(in_maps):
    fn, in_names, out_names, out_avals, zero_outs = _get_runner()
    concat_in = _concat_inputs(in_maps, in_names)
    concat_zeros = [np.zeros((NCORES * z.shape[0], *z.shape[1:]), z.dtype)
                    for z in zero_outs]
    out_arrs = fn(*concat_in, *concat_zeros)
    return out_arrs, out_names, out_avals


def kernel(unaries, rgb):
    in_maps = _host_prepare(unaries, rgb)
    out_arrs, out_names, out_avals = _run(in_maps)
    qi = out_names.index("qout")
    q = np.asarray(out_arrs[qi]).reshape(NCORES, S, L).reshape(N, L)
    return np.ascontiguousarray(q[None].astype(np.float32))


def time_kernel(unaries, rgb, iters=20):
    """Steady-state per-call wall time of the compiled 8-core executable,
    with inputs pre-staged on device."""
    import time as _time
    import jax
    in_maps = _host_prepare(unaries, rgb)
    fn, in_names, out_names, out_avals, zero_outs = _get_runner()
    concat_in = _concat_inputs(in_maps, in_names)

    def once():
        concat_zeros = [np.zeros((NCORES * z.shape[0], *z.shape[1:]), z.dtype)
                        for z in zero_outs]
        outs = fn(*concat_in, *concat_zeros)
        jax.block_until_ready(outs)
        return outs

    once()  # warm
    times = []
    for _ in range(iters):
        t0 = _time.perf_counter()
        once()
        times.append(_time.perf_counter() - t0)
    return min(times), sorted(times)[len(times) // 2]


# revision 71
# speedup vs baseline: 2.0293x; 1.0205x over previous
"""Trainium2 Bass kernel: dense-CRF mean-field layer (96x96 image, 21 labels).

Strategy (8 NeuronCores, row-sharded, K-stationary form):
  * Bilateral kernel K_bl [N,N] is built once on-device (fused feature matmul
    + exp) in bf16 and stays SBUF-resident per core as its [all j, own i]
    slice.
  * The per-iteration bilateral message uses K_bl tiles as the STATIONARY
    matmul operand and streams q chunks [128, 22] as the moving operand:
    out[96 own-pixels, 22] accumulates over 72 j-chunks.  Output is
    pixel-major, so the softmax/combine chain runs directly on [x, y, l]
    tiles with no transposes.
  * Spatial kernel is separable: y-blur is done with t1 (x-blurred q from the
    previous iteration, all-gathered) as the stationary operand per label,
    x-blur per own row after the softmax.  W_SPATIAL/norm folded into the
    host-prepared blur matrices.
  * Per iteration the new q is written straight into the all-gather payload
    (6 partition-shift DMA pieces), together with the x-blurred t1.
"""
import sys
sys.path.insert(0, "/opt/trn_rl_repo")
import os
import numpy as np
import ml_dtypes

H = W = 96
N = H * W                  # 9216
L = 21
LE = L + 1                 # 22 channels (21 labels + norm channel)
ALPHA, BETA, GAMMA = 80.0, 13.0, 3.0
W_SPATIAL, W_BILATERAL = 3.0, 10.0
NUM_ITERATIONS = 5
NCORES = 8
S = N // NCORES            # 1152 rows per core
YPC = H // NCORES          # 12 image rows per core
CH = N // 128              # 72 chunks of 128 rows (global j)
KCOLS = CH * S             # 82944 K_bl sbuf columns (bf16)
QCOLS = CH * LE            # 1584
PAYQ_F32 = 128 * 9 * LE // 2   # 12672 f32 slots holding the bf16 q-part
PAYT = S * LE // 2             # 12672 f32 slots holding the bf16 t1 part
PAY = PAYQ_F32 + PAYT          # 25344
ONESV = 0.1                # q norm-channel value => reciprocal gives 10/norm

# Schraudolph bf16 exp: bits_u16 = trunc(A_SCH * max(x + SH_SCH, 0)),
# bitcast as bf16 ~= exp(x) (max rel err 3.3%, C=5 fitted numerically)
A_SCH = 128.0 / 0.6931471805599453
SH_SCH = (16256.0 - 5.0) / A_SCH

# partition-shift piece groups for the 96->128 repack of q into the payload:
# maps qyb[x0:x0+n, yi, yo, l] -> q128[p0:p0+n, ao, yi, l]  (ai == yi)
# constraint: yo*96 == ao*128 - x0 + p0
QPIECES = ((0, 96, 0, 0, 0),
           (0, 96, 32, 3, 2),
           (0, 32, 96, 1, 0),
           (32, 64, 0, 1, 1),
           (0, 64, 64, 2, 1),
           (64, 32, 0, 2, 2))

LAST_EXEC_NS = None
_CACHE = {}


def _build_bass(sim1=False):
    """Build the kernel. sim1=True builds a single-core variant where the
    AllGather is replaced by 8 local DRAM copies (for TimelineSim analysis)."""
    key = "nc_sim1" if sim1 else "nc"
    if key in _CACHE:
        return _CACHE[key]
    import concourse.bass as bass  # noqa: F401
    from concourse import bacc
    import concourse.mybir as mybir
    import concourse.tile as tile

    f32 = mybir.dt.float32
    bf16 = mybir.dt.bfloat16
    AF = mybir.ActivationFunctionType
    OP = mybir.AluOpType
    AX = mybir.AxisListType

    dbg = bool(int(os.environ.get("CRF_DEBUG", "0"))) and not sim1
    nc = bacc.Bacc("TRN2", target_bir_lowering=False, debug=False,
                   num_devices=1 if sim1 else NCORES)

    featL_d = nc.dram_tensor("featL", [21, N], bf16, kind="ExternalInput")
    featR_d = nc.dram_tensor("featR", [21, S], bf16, kind="ExternalInput")
    uSB_d = nc.dram_tensor("uSB", [W, YPC * L], f32, kind="ExternalInput")
    Ax_d = nc.dram_tensor("Ax", [W, W], bf16, kind="ExternalInput")
    Ay_d = nc.dram_tensor("Ay", [H, YPC], bf16, kind="ExternalInput")
    qsb0_d = nc.dram_tensor("qsb0", [128, QCOLS], bf16, kind="ExternalInput")
    t1f0_d = nc.dram_tensor("t1f0", [H, LE * W], bf16, kind="ExternalInput")
    qout_d = nc.dram_tensor("qout", [S, L], f32, kind="ExternalOutput")
    if dbg:
        dbg_kbl = nc.dram_tensor("dbg_kbl", [128, S], bf16, kind="ExternalOutput")
        dbg_pbl = nc.dram_tensor("dbg_pbl", [W, YPC * LE], f32, kind="ExternalOutput")
        dbg_v = nc.dram_tensor("dbg_v", [W, YPC * L], f32, kind="ExternalOutput")
        dbg_lg = nc.dram_tensor("dbg_lg", [W, YPC * L], f32, kind="ExternalOutput")
        dbg_qy = nc.dram_tensor("dbg_qy", [W, YPC * L], f32, kind="ExternalOutput")
        dbg_t1t = nc.dram_tensor("dbg_t1t", [LE, S], bf16, kind="ExternalOutput")

    # combine groups: rows with y % 4 == yo finish together so the payload
    # piece(s) for that yo can fire while later groups still accumulate
    YGROUPS = [(yo, [yo, yo + 4, yo + 8]) for yo in range(4)]

    with tile.TileContext(nc) as tc:
        with (
            tc.tile_pool(name="const", bufs=1) as constp,
            tc.tile_pool(name="kbl", bufs=1) as kblp,
            tc.tile_pool(name="work", bufs=1) as work,
            tc.tile_pool(name="dram", bufs=2, space="DRAM") as dram,
        ):
            Ax = constp.tile([W, W], bf16)
            Ay = constp.tile([H, YPC], bf16)
            uSB = constp.tile([W, YPC * L], f32)
            Kbl = kblp.tile([128, KCOLS], bf16)
            qsb = work.tile([128, QCOLS], bf16, tag="qsb", bufs=2)
            t1full = work.tile([H, LE * W], bf16, tag="t1full", bufs=2)

            # ---------- precompute K_bl = exp(-||g_i - g_j||^2 / 2) ----------
            # Elementwise exp split across ACT (table exp) and DVE/Pool
            # (Schraudolph bf16-bitcast exp) over 512-col PSUM windows; the
            # 8-deep window ring lets all three engines run concurrently.
            u16 = mybir.dt.uint16
            # A: ACT table-exp from PSUM.  P: DVE shift+clamp PSUM->SBUF f32,
            # then Pool scale+u16-convert SBUF->SBUF (GPSIMD can't touch
            # PSUM).  D: DVE does both steps.
            WPAT = "APAPAPAPAPAPAPAPAPAPDAAAAAA"  # ACT/Pool/DVE 3-way split
            with (
                tc.tile_pool(name="pre_sb", bufs=2) as pre_sb,
                tc.tile_pool(name="stg", bufs=2) as stgp,
                tc.tile_pool(name="featRp", bufs=1) as featRp,
                tc.tile_pool(name="pre_ps", bufs=4, space="PSUM") as pre_ps,
            ):
                featR = featRp.tile([21, S], bf16)
                nc.sync.dma_start(featR[:], featR_d[:])
                flb, flb_idx = None, -1
                WIN = 1024
                NW = KCOLS // WIN
                for wdx in range(NW):
                    if wdx == 1:
                        # late-need loads, queued behind featR + first flb
                        nc.sync.dma_start(qsb[:], qsb0_d[:])
                        nc.sync.dma_start(t1full[:], t1f0_d[:])
                    if wdx == 2:
                        nc.sync.dma_start(Ax[:], Ax_d[:])
                        nc.sync.dma_start(Ay[:], Ay_d[:])
                        nc.sync.dma_start(uSB[:], uSB_d[:])
                    g0 = wdx * WIN
                    d2 = pre_ps.tile([128, WIN], f32, tag="d2")
                    cuts = sorted({g0, g0 + WIN}
                                  | set(range((g0 // 512 + 1) * 512,
                                              g0 + WIN, 512))
                                  | set(range((g0 // S + 1) * S,
                                              g0 + WIN, S)))
                    for a, b in zip(cuts[:-1], cuts[1:]):
                        ch = a // S
                        if ch // 8 != flb_idx:
                            flb_idx = ch // 8
                            flb = pre_sb.tile([21, 1024], bf16, tag="fl")
                            nc.sync.dma_start(
                                flb[:],
                                featL_d[:, flb_idx * 1024:(flb_idx + 1) * 1024])
                        nc.tensor.matmul(
                            d2[:, a - g0:b - g0],
                            flb[:, (ch % 8) * 128:(ch % 8 + 1) * 128],
                            featR[:, a - ch * S:b - ch * S],
                            start=True, stop=True)
                    e = WPAT[wdx % len(WPAT)]
                    if e == "A":
                        nc.scalar.activation(Kbl[:, g0:g0 + WIN],
                                             d2[:, 0:WIN], AF.Exp)
                    elif e == "D":
                        nc.vector.tensor_scalar(d2[:, 0:WIN], d2[:, 0:WIN],
                                                SH_SCH, 0.0,
                                                op0=OP.add, op1=OP.max)
                        nc.vector.tensor_scalar(
                            Kbl[:, g0:g0 + WIN].bitcast(u16),
                            d2[:, 0:WIN], A_SCH, None, op0=OP.mult)
                    else:
                        # DVE shifts/clamps PSUM->SBUF, Pool scales and
                        # converts SBUF->SBUF (GPSIMD cannot access PSUM)
                        stg = stgp.tile([128, WIN], f32, tag="stg")
                        nc.vector.tensor_scalar(stg[:], d2[:, 0:WIN],
                                                SH_SCH, 0.0,
                                                op0=OP.add, op1=OP.max)
                        nc.gpsimd.tensor_scalar(
                            Kbl[:, g0:g0 + WIN].bitcast(u16),
                            stg[:], A_SCH, None, op0=OP.mult)

            if dbg:
                nc.sync.dma_start(dbg_kbl.ap(), Kbl[:, 0:S])

            # ---------- mean-field iterations ----------
            psBL_ctx = tc.tile_pool(name="psBL", bufs=1, space="PSUM")
            psBL = psBL_ctx.__enter__()
            psSP_ctx = tc.tile_pool(name="psSP", bufs=1, space="PSUM")
            psSP = psSP_ctx.__enter__()
            psXB_ctx = tc.tile_pool(name="psXB", bufs=2, space="PSUM")
            psXB = psXB_ctx.__enter__()
            psWM_ctx = tc.tile_pool(name="psWM", bufs=1, space="PSUM")
            psWM = psWM_ctx.__enter__()

            def pe_warm_fillers(n, dep_kbl=False):
                """Dummy 512-col matmuls that keep the tensor engine's
                p-state ramp alive across DMA-bound stretches.  With
                dep_kbl, filler k reads a late K_bl window so the stream
                paces itself to the end of the build."""
                wm = psWM.tile([LE, 512], f32, tag="warm")
                for k in range(n):
                    w = (NW - n + k) if dep_kbl else (k % 64)
                    nc.tensor.matmul(wm[:], qsb[:, 0:LE],
                                     Kbl[:, w * 512:(w + 1) * 512],
                                     start=True, stop=True)

            # pe_warm_fillers(12, dep_kbl=True)
            qag_prev = None
            for it in range(NUM_ITERATIONS):
                last = it == NUM_ITERATIONS - 1
                if it > 0:
                    qsb = work.tile([128, QCOLS], bf16, tag="qsb", bufs=2)
                    t1full = work.tile([H, LE * W], bf16, tag="t1full",
                                       bufs=2)
                    TB = NCORES * PAYQ_F32

                    def t1recv(r, eng):
                        tsrc = (qag_prev[0:1, TB + r * PAYT:
                                         TB + (r + 1) * PAYT].bitcast(bf16)
                                .rearrange("a b -> (a b)")
                                .rearrange("(l y x) -> y l x",
                                           l=LE, y=YPC, x=W))
                        tdst = (t1full[r * YPC:(r + 1) * YPC, :]
                                .rearrange("y (l x) -> y l x", l=LE, x=W))
                        eng.dma_start(tdst, tsrc)
                    t1recv(0, nc.sync)
                    for h in (0, 1):
                        qsrc = (qag_prev[0:1, h * 4 * PAYQ_F32:
                                         (h + 1) * 4 * PAYQ_F32]
                                .bitcast(bf16)
                                .rearrange("a (r p c) -> p (a r) c",
                                           r=4, p=128))
                        nc.sync.dma_start(
                            qsb[:].rearrange("p (r c) -> p r c", r=NCORES)
                            [:, h * 4:(h + 1) * 4], qsrc)
                    t1recv(1, nc.sync)
                    for r in range(2, NCORES):
                        t1recv(r, nc.scalar if r % 2 else nc.gpsimd)

                # spatial y-blur: per label, strided t1 slice stationary
                sp = psSP.tile([W, L * YPC], f32, tag="sp", bufs=2)
                for lb in range(L):
                    nc.tensor.matmul(sp[:, lb * YPC:(lb + 1) * YPC],
                                     t1full[:, lb * W:(lb + 1) * W], Ay[:],
                                     start=True, stop=True)
                v = work.tile([W, YPC * L], f32, tag="v", bufs=2)
                v3 = v[:].rearrange("x (y l) -> x y l", l=L)
                nc.vector.tensor_tensor(
                    v3, uSB[:].rearrange("x (y l) -> x y l", l=L),
                    sp[:].rearrange("x (l y) -> x y l", l=L), OP.add)
                pbl = psBL.tile([W, YPC * LE], f32, tag="pbl", bufs=2)

                lg = work.tile([W, YPC * L], f32, tag="lg", bufs=2)
                lg3 = lg[:].rearrange("x (y l) -> x y l", l=L)
                qy = work.tile([W, YPC * L], f32, tag="qy", bufs=2)
                qy3 = qy[:].rearrange("x (y l) -> x y l", l=L)
                ssum = work.tile([W, YPC], f32, tag="ssum", bufs=2)
                if it == 0:
                    # the bilateral norm column is sum_j K[j,i] * 0.1 --
                    # constant across iterations, so its reciprocal is
                    # computed once and reused
                    rec = work.tile([W, YPC], f32, tag="rec", bufs=1)
                if last:
                    qyf = work.tile([W, YPC * LE], f32, tag="qyf")
                    qyf3 = qyf[:].rearrange("x (y l) -> x y l", l=LE)
                else:
                    # per-group q tiles keep the payload-piece DMA deps
                    # narrow (whole-tile tracking would defer every piece
                    # to the last group's softmax)
                    qybs = [work.tile([W, 3 * LE], bf16, tag=f"qyb{g}",
                                      name=f"qyb{g}", bufs=2)
                            for g in range(4)]
                    t1X = work.tile([LE, S], bf16, tag="t1X", bufs=2)
                    pl = dram.tile([1, PAY], f32, tag="pl")
                    plq = (pl[0:1, 0:PAYQ_F32].bitcast(bf16)
                           .rearrange("a (p ai ao l) -> (a p) ao ai l",
                                      p=128, ai=3, ao=3))
                pbl3 = pbl[:].rearrange("x (y l) -> x y l", l=LE)

                def emit_xblur(yo):
                    # x-blur rows y%4==yo (new q stationary, Ax moving)
                    gsl = slice(yo, yo + 9, 4)
                    xb = psXB.tile([LE, 3 * W], f32, tag="xb")
                    for k, r in enumerate((yo, yo + 4, yo + 8)):
                        nc.tensor.matmul(xb[:, k * W:(k + 1) * W],
                                         qybs[yo][:, k * LE:(k + 1) * LE],
                                         Ax[:], start=True, stop=True)
                    nc.scalar.copy(
                        t1X[:].rearrange("l (y x) -> l y x", x=W)[:, gsl],
                        xb[:].rearrange("l (k x) -> l k x", x=W))

                for (yo, ys) in YGROUPS:
                    for r in ys:
                        for ch in range(CH):
                            nc.tensor.matmul(
                                pbl[:, r * LE:(r + 1) * LE],
                                Kbl[:, ch * S + r * W: ch * S + (r + 1) * W],
                                qsb[:, ch * LE:(ch + 1) * LE],
                                start=(ch == 0), stop=(ch == CH - 1))
                    # previous group's x-blur: emitted here so the PE never
                    # waits on the previous group's softmax chain
                    if yo > 0 and not last:
                        emit_xblur(yo - 1)
                    # combine + softmax for this group's 3 rows
                    gsl = slice(yo, yo + 9, 4)  # rows yo, yo+4, yo+8
                    if it == 0:
                        nc.vector.reciprocal(rec[:, gsl][:, :, None],
                                             pbl3[:, gsl, L:LE])
                    nc.vector.tensor_tensor(
                        lg3[:, gsl], pbl3[:, gsl, 0:L],
                        rec[:, gsl][:, :, None].to_broadcast([W, 3, L]),
                        OP.mult)
                    nc.vector.tensor_tensor(lg3[:, gsl], lg3[:, gsl],
                                            v3[:, gsl], OP.add)
                    nc.scalar.activation(qy3[:, gsl], lg3[:, gsl], AF.Exp)
                    nc.vector.reduce_sum(ssum[:, gsl], qy3[:, gsl], axis=AX.X)
                    nc.vector.reciprocal(ssum[:, gsl], ssum[:, gsl])
                    qt = (qyf3[:, gsl] if last
                          else qybs[yo][:].rearrange("x (k l) -> x k l", l=LE))
                    nc.vector.tensor_tensor(
                        qt[:, :, 0:L], qy3[:, gsl],
                        ssum[:, gsl][:, :, None].to_broadcast([W, 3, L]),
                        OP.mult)
                    if last:
                        continue
                    if it < 2:
                        nc.vector.memset(qt[:, :, L:LE], ONESV)
                    # payload piece(s) for this yo straight into DRAM;
                    # the last group's piece goes on SP so it lands just
                    # before the q-section gather on the same queue
                    for (x0, n, p0, pyo, ao) in QPIECES:
                        if pyo == yo:
                            eng = nc.sync if yo == 3 else nc.gpsimd
                            eng.dma_start(
                                plq[p0:p0 + n, ao],
                                qybs[yo][x0:x0 + n, :]
                                .rearrange("x (k l) -> x k l", l=LE))
                if not last:
                    emit_xblur(3)

                if dbg and it == 0:
                    nc.sync.dma_start(dbg_pbl.ap(), pbl[:])
                    nc.sync.dma_start(dbg_lg.ap(), lg[:])
                if dbg and it == 1:
                    nc.sync.dma_start(dbg_qy.ap(), qy[:])
                    nc.sync.dma_start(dbg_t1t.ap(), t1X[:])

                if last:
                    nc.sync.dma_start(
                        qout_d.ap().rearrange("(y x) l -> x y l", x=W),
                        qyf3[:, :, 0:L])
                    continue

                # t1 payload part (layout (l, y, x) per core)
                nc.scalar.dma_start(
                    pl[0:1, PAYQ_F32:PAY].bitcast(bf16)
                      .rearrange("a (l c) -> (a l) c", l=LE),
                    t1X[:])
                # AllGather split by payload section into a contiguous
                # section-major gather buffer [q sections | t1 sections]:
                # the q gather only waits on the q pieces, so the qsb chain
                # starts while the t1 payload is still being written
                qag = dram.tile([1, NCORES * PAY], f32, tag="qag")
                if sim1:
                    nc.sync.dma_start(
                        qag[0:1, 0:NCORES * PAYQ_F32],
                        pl[0:1, 0:PAYQ_F32].to_broadcast([NCORES, PAYQ_F32]))
                    nc.scalar.dma_start(
                        qag[0:1, NCORES * PAYQ_F32:],
                        pl[0:1, PAYQ_F32:PAY].to_broadcast([NCORES, PAYT]))
                else:
                    nc.gpsimd.collective_compute(
                        "AllGather", OP.bypass,
                        replica_groups=[list(range(NCORES))],
                        ins=[pl[0:1, 0:PAYQ_F32].opt()],
                        outs=[qag[0:1, 0:NCORES * PAYQ_F32].opt()])
                    nc.gpsimd.collective_compute(
                        "AllGather", OP.bypass,
                        replica_groups=[list(range(NCORES))],
                        ins=[pl[0:1, PAYQ_F32:PAY].opt()],
                        outs=[qag[0:1, NCORES * PAYQ_F32:].opt()])
                qag_prev = qag
            psWM_ctx.__exit__(None, None, None)
            psXB_ctx.__exit__(None, None, None)
            psSP_ctx.__exit__(None, None, None)
            psBL_ctx.__exit__(None, None, None)

    nc.compile()
    _CACHE[key] = nc
    return nc


def _host_prepare(unaries, rgb):
    u = np.asarray(unaries, np.float32).reshape(N, L)
    c = np.asarray(rgb, np.float32).reshape(N, 3)

    ys, xs = np.meshgrid(np.arange(H, dtype=np.float64),
                         np.arange(W, dtype=np.float64), indexing="ij")
    pos = np.stack([ys.ravel(), xs.ravel()], -1)            # [N, 2]
    g = np.concatenate([c.astype(np.float64) / BETA, pos / ALPHA], 1)
    g = g - g.mean(0, keepdims=True)
    sq = (g * g).sum(1)
    ones = np.ones(N, np.float64)
    L7 = np.concatenate([g.T, ones[None], (-0.5 * sq)[None]], 0)  # [7, N] j
    R7 = np.concatenate([g.T, (-0.5 * sq)[None], ones[None]], 0)  # [7, N] i
    bfd = ml_dtypes.bfloat16
    Lhi = L7.astype(bfd)
    Llo = (L7 - Lhi.astype(np.float64)).astype(bfd)
    Rhi = R7.astype(bfd)
    Rlo = (R7 - Rhi.astype(np.float64)).astype(bfd)
    # dot = Lhi.Rhi + Lhi.Rlo + Llo.Rhi  (Llo.Rlo dropped, ~1e-3)
    featL = np.ascontiguousarray(np.concatenate([Lhi, Lhi, Llo], 0))  # [21,N]
    featR = np.ascontiguousarray(np.concatenate([Rhi, Rlo, Rhi], 0))  # [21,N]

    d = np.arange(W, dtype=np.float64)
    A = np.exp(-(d[:, None] - d[None, :]) ** 2 / (2.0 * GAMMA * GAMMA))
    nvec = A.sum(0)
    Ax = np.ascontiguousarray((A / nvec[None, :]).astype(ml_dtypes.bfloat16))

    um = u.max(1, keepdims=True)
    e = np.exp(u - um)
    q0 = e / e.sum(1, keepdims=True)
    q0e = np.concatenate([q0, np.full((N, 1), ONESV, np.float32)], 1)  # [N,22]
    qsb0 = np.ascontiguousarray(
        q0e.reshape(CH, 128, LE).transpose(1, 0, 2).reshape(128, QCOLS)
    ).astype(ml_dtypes.bfloat16)

    q3 = q0e.reshape(H, W, LE).astype(np.float64)
    t1 = np.einsum("Xx,yXl->ylx", A / nvec[None, :], q3)      # [96, 22, 96]
    t1f0 = np.ascontiguousarray(t1.reshape(H, LE * W).astype(ml_dtypes.bfloat16))

    in_maps = []
    for core in range(NCORES):
        rows = slice(core * S, (core + 1) * S)
        uSB_c = np.ascontiguousarray(
            u[rows].reshape(YPC, W, L).transpose(1, 0, 2).reshape(W, YPC * L))
        yc = slice(core * YPC, (core + 1) * YPC)
        Ay_c = np.ascontiguousarray(
            (A[:, yc] * (W_SPATIAL / nvec[yc])[None, :]).astype(ml_dtypes.bfloat16))
        in_maps.append({
            "featL": featL,
            "featR": np.ascontiguousarray(featR[:, rows]),
            "uSB": uSB_c,
            "Ax": Ax,
            "Ay": Ay_c,
            "qsb0": qsb0,
            "t1f0": t1f0,
        })
    return in_maps


def _get_runner():
    """Compile once; return (fn, in_names, out_names) where fn maps
    concatenated global numpy inputs -> list of per-core output dicts."""
    if "runner" in _CACHE:
        return _CACHE["runner"]
    import jax
    from jax.sharding import Mesh, PartitionSpec
    from jax.experimental.shard_map import shard_map
    import concourse.mybir as mybir
    from concourse import bass2jax

    nc = _build_bass()
    bass2jax.install_neuronx_cc_hook()

    partition_name = (nc.partition_id_tensor.name
                      if nc.partition_id_tensor else None)
    in_names, out_names, out_avals, zero_outs = [], [], [], []
    for alloc in nc.m.functions[0].allocations:
        if not isinstance(alloc, mybir.MemoryLocationSet):
            continue
        name = alloc.memorylocations[0].name
        if alloc.kind == "ExternalInput":
            if name != partition_name:
                in_names.append(name)
        elif alloc.kind == "ExternalOutput":
            shape = tuple(alloc.tensor_shape)
            dtype = mybir.dt.np(alloc.dtype)
            out_names.append(name)
            out_avals.append(jax.core.ShapedArray(shape, dtype))
            zero_outs.append(np.zeros(shape, dtype))
    n_params = len(in_names)
    all_in_names = list(in_names) + list(out_names)
    if partition_name is not None:
        all_in_names.append(partition_name)

    def _body(*args):
        operands = list(args)
        if partition_name is not None:
            operands.append(bass2jax.partition_id_tensor())
        outs = bass2jax._bass_exec_p.bind(
            *operands,
            out_avals=tuple(out_avals),
            in_names=tuple(all_in_names),
            out_names=tuple(out_names),
            lowering_input_output_aliases=(),
            sim_require_finite=False,
            sim_require_nnan=False,
            nc=nc,
        )
        return tuple(outs)

    devices = jax.devices()[:NCORES]
    mesh = Mesh(np.asarray(devices), ("core",))
    n_outs = len(out_names)
    in_specs = (PartitionSpec("core"),) * (n_params + n_outs)
    out_specs = (PartitionSpec("core"),) * n_outs
    donate = tuple(range(n_params, n_params + n_outs))
    fn = jax.jit(
        shard_map(_body, mesh=mesh, in_specs=in_specs, out_specs=out_specs,
                  check_rep=False),
        donate_argnums=donate, keep_unused=True)
    _CACHE["runner"] = (fn, in_names, out_names, out_avals, zero_outs)
    return _CACHE["runner"]


def _concat_inputs(in_maps, in_names):
    return [np.concatenate([np.asarray(in_maps[c][nm]) for c in range(NCORES)],
                           axis=0) for nm in in_names]


def _run(in_maps):
    fn, in_names, out_names, out_avals, zero_outs = _get_runner()
    concat_in = _concat_inputs(in_maps, in_names)
    concat_zeros = [np.zeros((NCORES * z.shape[0], *z.shape[1:]), z.dtype)
                    for z in zero_outs]
    out_arrs = fn(*concat_in, *concat_zeros)
    return out_arrs, out_names, out_avals


def kernel(unaries, rgb):
    in_maps = _host_prepare(unaries, rgb)
    out_arrs, out_names, out_avals = _run(in_maps)
    qi = out_names.index("qout")
    q = np.asarray(out_arrs[qi]).reshape(NCORES, S, L).reshape(N, L)
    return np.ascontiguousarray(q[None].astype(np.float32))


def time_kernel(unaries, rgb, iters=20):
    """Steady-state per-call wall time of the compiled 8-core executable,
    with inputs pre-staged on device."""
    import time as _time
    import jax
    in_maps = _host_prepare(unaries, rgb)
    fn, in_names, out_names, out_avals, zero_outs = _get_runner()
    concat_in = _concat_inputs(in_maps, in_names)

    def once():
        concat_zeros = [np.zeros((NCORES * z.shape[0], *z.shape[1:]), z.dtype)
                        for z in zero_outs]
        outs = fn(*concat_in, *concat_zeros)
        jax.block_until_ready(outs)
        return outs

    once()  # warm
    times = []
    for _ in range(iters):
        t0 = _time.perf_counter()
        once()
        times.append(_time.perf_counter() - t0)
    return min(times), sorted(times)[len(times) // 2]
